# revision 61
# baseline (speedup 1.0000x reference)
"""Trainium2 Bass kernel for CNN-encoder + attention-LSTM captioner + vocab FC.

Sharding: pure data-parallel over batch (16 images -> 8 cores x 2 images).
All weights replicated; no collectives. Host slices inputs / concatenates outputs.

Layout conventions (per core, B=2 local images, T=32 steps):
  - tokens are indexed p = t*2 + b  (t-major) so each LSTM step reads a
    contiguous partition pair from the batched precompute.
  - recurrent state h is kept transposed ([HID, 2] chunks) in outsT so it can
    feed the next step's matmul lhsT directly and the final FC lhsT.
"""

import os
import numpy as np

os.environ.setdefault("MYCRO_LOCAL_CACHE", "1")

HID = 640
VOCAB = 10000
T = 32
BL = 2            # local batch per core
NTOK = T * BL     # 64
NCORES = 8

F32 = None  # set lazily (mybir.dt.float32)


class _PhaseExit(Exception):
    def __init__(self, tc):
        self.tc = tc

_NC_CACHE = {}
PHASE_MARKS = []   # (phase_name, inst_count_at_phase_end) recorded during build


def _gate_perm():
    # reference gate order [i, f, g, o] -> kernel order [i, f, o, g]
    return np.concatenate([
        np.arange(0, 1280),          # i, f
        np.arange(1920, 2560),       # o
        np.arange(1280, 1920),       # g
    ])


def build_bass(upto=None):
    import os
    upto = upto or os.environ.get("KERNEL_UPTO", "all")
    import concourse.bass as bass
    from concourse import bacc
    import concourse.tile_sem_assignment as tsa
    # Cap HWDGE sem lanes so pool-transition fan-ins stay under the
    # per-instruction sync-wait slot limits in walrus codegen.
    tsa.NUM_HWDGE_SEMS = 4
    import concourse.mybir as mybir
    import concourse.tile as tile
    from concourse.masks import make_identity

    f32 = mybir.dt.float32
    i32 = mybir.dt.int32
    AF = mybir.ActivationFunctionType
    ALU = mybir.AluOpType
    AX = mybir.AxisListType

    nc = bacc.Bacc(None)
    bf16 = mybir.dt.bfloat16
    f8 = mybir.dt.float8e4

    PHASE_MARKS.clear()

    def mark(name):
        PHASE_MARKS.append((name, len(nc.inst_map)))

    def mm(out, lhsT, rhs, **kw):
        nc.tensor.matmul(out=out, lhsT=lhsT, rhs=rhs, **kw)

    # ---------------- DRAM parameters ----------------
    img_d = nc.declare_dram_parameter("img", [55, 224 * 224], bf16, isOutput=False)
    caps_d = nc.declare_dram_parameter("caps", [NTOK, 1], i32, isOutput=False)
    w1s_d = nc.declare_dram_parameter("w1s", [55, 128], bf16, isOutput=False)
    cb2_d = nc.declare_dram_parameter("cb2t", [128, 1], f32, isOutput=False)
    w2p_d = nc.declare_dram_parameter("w2p", [2, 128, 3, 128], bf16, isOutput=False)
    w2s_d = nc.declare_dram_parameter("w2s", [128, 3, 128], bf16, isOutput=False)
    w3t9_d = nc.declare_dram_parameter("w3t9", [9, 128, 256], bf16, isOutput=False)
    w4t9_d = nc.declare_dram_parameter("w4t9", [128, 4, 2, 9, 128], bf16, isOutput=False)
    cb3_d = nc.declare_dram_parameter("cb3t", [128, 2], f32, isOutput=False)
    cb4_d = nc.declare_dram_parameter("cb4t", [128, 4], f32, isOutput=False)
    encw_d = nc.declare_dram_parameter("encwt", [4, 128, HID], f32, isOutput=False)
    encb_d = nc.declare_dram_parameter("encbt", [128, 5], f32, isOutput=False)
    emb_d = nc.declare_dram_parameter("emb", [VOCAB, HID], bf16, isOutput=False)
    attnw_d = nc.declare_dram_parameter("attnwt", [10, 128, HID], bf16, isOutput=False)
    attnb_d = nc.declare_dram_parameter("attnb", [1, HID], bf16, isOutput=False)
    wih_d = nc.declare_dram_parameter("wiht", [20, 128, 10, 128], bf16, isOutput=False)
    whh_d = nc.declare_dram_parameter("whht", [5, 128, 4 * HID], bf16, isOutput=False)
    bgate_d = nc.declare_dram_parameter("bgate", [1, 4 * HID], bf16, isOutput=False)
    fcw_d = nc.declare_dram_parameter("fcwt", [5, 128, VOCAB], bf16, isOutput=False)
    fcb_d = nc.declare_dram_parameter("fcb", [1, VOCAB], bf16, isOutput=False)
    bsel_d = nc.declare_dram_parameter("bsel", [BL, NTOK], f32, isOutput=False)
    # logits stored transposed + group-blocked: [group, vocab_row, chunk, tok]
    NVC = 80          # vocab chunks
    VC = VOCAB // NVC  # 125 vocab rows per chunk
    logits_d = nc.declare_dram_parameter("logits", [10, VC, 8, NTOK], f32,
                                         isOutput=True)

    try:
      with tile.TileContext(nc) as tc:
        # ---------------- persistent constants ----------------
        cpool = tc.alloc_tile_pool(name="const", bufs=1)
        # pool for all DMA-written tiles: never released mid-kernel so that
        # SBUF zone reuse never makes compute ops wait on DMA queue sems
        dmapool = tc.alloc_tile_pool(name="dmat", bufs=1)
        ident = cpool.tile([128, 128], f32)
        make_identity(nc, ident[:, :])
        identb = cpool.tile([128, 128], bf16)
        make_identity(nc, identb[:, :])
        ones64 = cpool.tile([1, 64], bf16)
        nc.gpsimd.memset(ones64[:, :], 1.0)
        bsel_sb = dmapool.tile([BL, NTOK], f32)
        nc.sync.dma_start(out=bsel_sb[:, :], in_=bsel_d[:, :])
        feat_sb = cpool.tile([128, 4, BL], f32)   # feat.T, K-chunked [128,4] per img

        w1s_sb = dmapool.tile([55, 128], bf16)
        nc.sync.dma_start(out=w1s_sb[:, :], in_=w1s_d[:, :])
        cb2_sb = dmapool.tile([128, 1], f32)
        nc.sync.dma_start(out=cb2_sb[:, :], in_=cb2_d[:, :])
        w2p_sb = dmapool.tile([128, 2, 3, 128], bf16)
        nc.sync.dma_start(out=w2p_sb[:, :, :, :],
                          in_=w2p_d[:, :, :, :].rearrange("i p t o -> p i t o"))
        w2s_sb = dmapool.tile([128, 3, 128], bf16)
        nc.sync.dma_start(out=w2s_sb[:, :, :], in_=w2s_d[:, :, :])
        w3_sb = dmapool.tile([128, 9, 256], bf16)
        nc.sync.dma_start(out=w3_sb[:, :, :], in_=w3t9_d[:, :, :].rearrange("t p o -> p t o"))
        cb3_sb = dmapool.tile([128, 2], f32)
        nc.sync.dma_start(out=cb3_sb[:, :], in_=cb3_d[:, :])
        cb4_sb = dmapool.tile([128, 4], f32)
        nc.sync.dma_start(out=cb4_sb[:, :], in_=cb4_d[:, :])

        # ---------------- conv tower ----------------
        # per-image padded intermediates; only borders need zeroing (interiors
        # are fully rewritten).
        padpool = tc.alloc_tile_pool(name="pads", bufs=1)
        # staging: [im0ch | im1ch] padded pool1 output + an extra always-zero
        # row 114 so the row+1-shifted dup copies need no edge memset
        x2s = padpool.tile([128, 115, 114], bf16)
        nc.vector.memset(x2s[:, 0, :], 0.0)
        nc.vector.memset(x2s[:, 113:115, :], 0.0)
        nc.vector.memset(x2s[:, :, 0], 0.0)
        nc.vector.memset(x2s[:, :, 113], 0.0)
        x2im = []
        for im in range(BL):
            x2t = padpool.tile([128, 114, 114], bf16)
            x2im.append(x2t)
        x3im = [None, None]
        x4im = [None, None]

        # ---- conv1 (3->64), both images stacked on partitions (M=64ch x 2im),
        # im2col K=54 + ones-row bias; pool chain: DVE xmax (psum evict),
        # Pool rowmax + relu ----
        _sc1 = nc.enter_named_scope("conv1", False)[0]
        c1pool = tc.alloc_tile_pool(name="c1", bufs=3)
        c1psum = tc.alloc_tile_pool(name="c1p", bufs=3, space="PSUM")
        R = 16
        for ch in range(224 // R):
            Y = R * ch
            rh = dmapool.tile([55, R * 224], bf16, tag="rh", bufs=2)
            nc.sync.dma_start(out=rh[:, :],
                              in_=img_d[:, Y * 224:(Y + R) * 224])
            rhv = rh.rearrange("p (r x) -> p r x", x=224)
            for q in range(4):
                ps = c1psum.tile([128, 2, 448], f32, padded_shape=[128, 2, 512],
                                 tag="ps")
                for s in range(2):
                    r0 = q * 4 + s * 2
                    mm(out=ps[:, s, :], lhsT=w1s_sb[:, :],
                       rhs=rhv[:, r0:r0 + 2, :], start=True, stop=True)
                # relu(maxpool) == max(even, relu(odd), rows): ACT relu-evicts
                # only the odd columns; DVE maxes them against the raw PSUM
                # evens (xm >= 0 always), then row-pools into x2s.
                a1 = c1pool.tile([128, 2, 224], bf16, tag="a1")
                nc.scalar.activation(a1[:, :, :], ps[:, :, 1:448:2], AF.Relu)
                xm = c1pool.tile([128, 2, 224], bf16, tag="xm")
                nc.vector.tensor_tensor(out=xm[:, :, :],
                                        in0=ps[:, :, 0:448:2],
                                        in1=a1[:, :, :], op=ALU.max)
                oy = (R * ch + 4 * q) // 2
                nc.vector.tensor_tensor(out=x2s[:, oy + 1:oy + 3, 1:113],
                                        in0=xm[:, :, 0:112],
                                        in1=xm[:, :, 112:224], op=ALU.max)
        w4_sb = dmapool.tile([128, 4, 2, 9, 128], bf16)
        nc.sync.dma_start(out=w4_sb[:, :, :, :, :], in_=w4t9_d[:, :, :, :, :])
        encw_sb = dmapool.tile([128, 4, HID], f32)
        nc.sync.dma_start(out=encw_sb[:, :, :], in_=encw_d[:, :, :].rearrange("k p o -> p k o"))
        encb_sb = dmapool.tile([128, 5], f32)
        nc.sync.dma_start(out=encb_sb[:, :], in_=encb_d[:, :])
        attnw_sb = dmapool.tile([128, 10, HID], bf16)
        nc.sync.dma_start(out=attnw_sb[:, :, :],
                          in_=attnw_d[:, :, :].rearrange("k p o -> p k o"))
        attnb_sb = dmapool.tile([1, HID], bf16)
        nc.sync.dma_start(out=attnb_sb[:, :], in_=attnb_d[:, :])
        whh_sb = dmapool.tile([128, 5, 4 * HID], bf16)
        nc.sync.dma_start(out=whh_sb[:, :, :],
                          in_=whh_d[:, :, :].rearrange("k p o -> p k o"))
        bgate_sb = dmapool.tile([1, 4 * HID], bf16)
        nc.sync.dma_start(out=bgate_sb[:, :], in_=bgate_d[:, :])
        idx_sb = dmapool.tile([NTOK, 1], i32)
        nc.sync.dma_start(out=idx_sb[:, :], in_=caps_d[:, :])
        e_sb = dmapool.tile([NTOK, HID], bf16)
        nc.gpsimd.indirect_dma_start(
            out=e_sb[:, :], out_offset=None,
            in_=emb_d[:, :],
            in_offset=bass.IndirectOffsetOnAxis(ap=idx_sb[:, :1], axis=0),
        )
        c1psum.release()
        c1pool.release()
        # assemble per-image tap-pair layouts from the staging tile:
        # x2im0 = [im0 | im0 shifted+1], x2im1 = [im1 shifted+1 | im1]
        for r0, r1 in ((0, 57), (57, 114)):
            nc.sync.dma_start(out=x2im[0][0:64, r0:r1, :],
                              in_=x2s[0:64, r0:r1, :])
            nc.sync.dma_start(out=x2im[0][64:128, r0:r1, :],
                              in_=x2s[0:64, r0 + 1:r1 + 1, :])
            nc.sync.dma_start(out=x2im[1][64:128, r0:r1, :],
                              in_=x2s[64:128, r0:r1, :])
            nc.sync.dma_start(out=x2im[1][0:64, r0:r1, :],
                              in_=x2s[64:128, r0 + 1:r1 + 1, :])
        nc.leave_named_scope("conv1", _sc1, False)
        mark("conv1")
        for im in range(BL):
            _sc = nc.enter_named_scope(f"conv_im{im}", False)[0]
            ipool = tc.alloc_tile_pool(name=f"img{im}", bufs=1)

            # ---- conv2 (64->128): 3 K=128 tap-pairs + 3 K=64 singles ----
            x3t = padpool.tile([128, 58, 58], bf16)
            nc.vector.memset(x3t[:, :, :], 0.0)
            x3im[im] = x3t
            x4t = padpool.tile([128, 2, 30, 30], bf16)
            nc.vector.memset(x4t[:, :, :, :], 0.0)
            x4im[im] = x4t
            c2psum = tc.alloc_tile_pool(name=f"c2p_{im}", bufs=3, space="PSUM")
            c2pool = tc.alloc_tile_pool(name=f"c2_{im}", bufs=2)
            sb = 0 if im == 0 else 64   # singles partition base for this image
            for tl in range(14):  # 8 output rows per tile
                ps = c2psum.tile([128, 2, 448], f32, padded_shape=[128, 2, 512], tag="ps")
                for s in range(2):
                    y0 = tl * 8 + s * 4
                    for kx in range(3):
                        mm(
                            out=ps[:, s, :], lhsT=w2p_sb[:, im, kx, :],
                            rhs=x2im[im][:, y0:y0 + 4, kx:kx + 112],
                            start=(kx == 0), stop=False,
                        )
                    for kx in range(3):
                        mm(
                            out=ps[:, s, :], lhsT=w2s_sb[sb:sb + 64, kx, :],
                            rhs=x2im[im][sb:sb + 64, y0 + 2:y0 + 6, kx:kx + 112],
                            start=False, stop=(kx == 2),
                        )
                a2 = c2pool.tile([128, 8, 112], bf16, tag="a2")
                nc.scalar.activation(
                    a2.rearrange("p (a y) x -> p a y x", a=2),
                    ps.rearrange("p a (y x) -> p a y x", x=112),
                    AF.Relu, bias=cb2_sb[:, 0:1])
                t2 = c2pool.tile([128, 8, 56], bf16, tag="t2")
                nc.vector.tensor_tensor(
                    out=t2[:, :, :], in0=a2[:, :, 0:112:2], in1=a2[:, :, 1:112:2],
                    op=ALU.max,
                )
                t2b = c2pool.tile([128, 4, 56], bf16, tag="t2b")
                nc.vector.tensor_tensor(
                    out=t2b[:, :, :], in0=t2[:, 0:8:2, :], in1=t2[:, 1:8:2, :],
                    op=ALU.max,
                )
                nc.vector.tensor_copy(
                    out=x3im[im][:, tl * 4 + 1:tl * 4 + 5, 1:57],
                    in_=t2b[:, :, :],
                )
            c2psum.release()
            c2pool.release()

            # ---- conv3 (128->256) K=128, bias via ACT evict, pool -> x4_pad ----
            c3psum = tc.alloc_tile_pool(name=f"c3p_{im}", bufs=3, space="PSUM")
            c3pool = tc.alloc_tile_pool(name=f"c3_{im}", bufs=2)
            for m in range(2):
                for tl in range(7):  # 8 output rows per tile
                    ps = c3psum.tile([128, 448], f32, padded_shape=[128, 512], tag="ps")
                    y0 = tl * 8
                    for ky in range(3):
                        for kx in range(3):
                            tap = ky * 3 + kx
                            rhs = x3im[im][:, y0 + ky:y0 + ky + 8, kx:kx + 56]
                            mm(
                                out=ps[:, :],
                                lhsT=w3_sb[:, tap, 128 * m:128 * (m + 1)],
                                rhs=rhs,
                                start=(tap == 0), stop=(tap == 8),
                            )
                    a3 = c3pool.tile([128, 8, 56], bf16, tag="a3")
                    nc.scalar.activation(
                        a3[:, :, :],
                        ps.rearrange("p (y x) -> p y x", x=56),
                        AF.Relu, bias=cb3_sb[:, m:m + 1])
                    t3 = c3pool.tile([128, 8, 28], bf16, tag="t3")
                    nc.vector.tensor_tensor(
                        out=t3[:, :, :], in0=a3[:, :, 0:56:2], in1=a3[:, :, 1:56:2],
                        op=ALU.max,
                    )
                    nc.vector.tensor_tensor(
                        out=x4im[im][:, m, tl * 4 + 1:tl * 4 + 5, 1:29],
                        in0=t3[:, 0:8:2, :], in1=t3[:, 1:8:2, :],
                        op=ALU.max,
                    )
            c3psum.release()
            c3pool.release()

            # ---- conv4 (256->512) K=256 (2 chunks), no pool; mean via accum_out ----
            c4psum = tc.alloc_tile_pool(name=f"c4p_{im}", bufs=3, space="PSUM")
            c4pool = tc.alloc_tile_pool(name=f"c4_{im}", bufs=2)
            msum = ipool.tile([128, 4, 2], f32)
            for m in range(4):
                w4m = w4_sb[:, m]
                ps = c4psum.tile([128, 2, 392], f32, padded_shape=[128, 2, 512], tag="ps")
                for s in range(2):
                    y0 = s * 14
                    first = True
                    for ky in range(3):
                        for kx in range(3):
                            tap = ky * 3 + kx
                            for k2 in range(2):
                                rhs = x4im[im][:, k2, y0 + ky:y0 + ky + 14, kx:kx + 28]
                                mm(
                                    out=ps[:, s, :],
                                    lhsT=w4m[:, k2, tap, :],
                                    rhs=rhs,
                                    start=first, stop=(tap == 8 and k2 == 1),
                                )
                                first = False
                a4 = c4pool.tile([128, 2, 392], bf16, tag="a4")
                for s in range(2):
                    nc.scalar.activation(a4[:, s, :], ps[:, s, :], AF.Relu,
                                         bias=cb4_sb[:, m:m + 1],
                                         accum_out=msum[:, m, s:s + 1])
            c4psum.release()
            c4pool.release()
            # feat.T[:, m] = (msum[:,m,0] + msum[:,m,1]) / 784
            tmpf = ipool.tile([128, 4], f32)
            nc.vector.tensor_tensor(out=tmpf[:, :], in0=msum[:, :, 0], in1=msum[:, :, 1],
                                    op=ALU.add)
            nc.vector.tensor_scalar_mul(feat_sb[:, :, im], tmpf[:, :], 1.0 / 784.0)
            ipool.release()
            nc.leave_named_scope(f"conv_im{im}", _sc, False)
            mark(f"conv_im{im}")

        padpool.release()

        if upto == "conv":
            raise _PhaseExit(tc)

        spool = tc.alloc_tile_pool(name="seq", bufs=1)
        NG = 4 * HID // 128   # 20 gate chunks: i=0-4, f=5-9, o=10-14, g=15-19
        # full w_ih prefetch in a scoped pool (freed before the FC weights land)
        wihpool = tc.alloc_tile_pool(name="wihp", bufs=1)
        wih_sb = wihpool.tile([128, NG, 10, 128], bf16)
        for g in range(4):
            nc.sync.dma_start(
                out=wih_sb[:, 5 * g:5 * (g + 1), :, :],
                in_=wih_d[5 * g:5 * (g + 1), :, :, :].rearrange(
                    "m p k o -> p m k o"))

        # ---------------- encoder linear: memory.T = enc_w @ feat.T + enc_b ----------------
        _sc_ea = nc.enter_named_scope("enc_attn", False)[0]
        scpool = tc.alloc_tile_pool(name="scratch", bufs=1)
        p1psum = tc.alloc_tile_pool(name="p1ps", bufs=1, space="PSUM")
        memT_ps = p1psum.tile([128, 5, BL], f32)
        for m in range(5):
            for k in range(4):
                nc.tensor.matmul(
                    out=memT_ps[:, m, :],
                    lhsT=encw_sb[:, k, 128 * m:128 * (m + 1)],
                    rhs=feat_sb[:, k, :],
                    start=(k == 0), stop=(k == 3),
                )
        memT_sb = spool.tile([128, 5, BL], f32)
        for m in range(5):
            nc.vector.tensor_scalar_add(memT_sb[:, m, :], memT_ps[:, m, :],
                                        encb_sb[:, m:m + 1])
        # memory non-transposed [2, 640]
        mem_ps = p1psum.tile([BL, HID], f32)
        for m in range(5):
            nc.tensor.transpose(out=mem_ps[:, 128 * m:128 * (m + 1)],
                                in_=memT_sb[:, m, :], identity=ident[:, :])
        mem_sb = scpool.tile([BL, HID], f32)
        nc.scalar.copy(mem_sb[:, :], mem_ps[:, :])

        # memory broadcast to all tokens [64, 640] via bsel matmul
        mexp_ps = p1psum.tile([NTOK, HID], f32)
        for n in range(2):
            sl = slice(512 * n, min(HID, 512 * (n + 1)))
            nc.tensor.matmul(out=mexp_ps[:, sl], lhsT=bsel_sb[:, :], rhs=mem_sb[:, sl],
                             start=True, stop=True)
        mexp_sb = scpool.tile([NTOK, HID], f32)
        nc.scalar.copy(mexp_sb[:, :], mexp_ps[:, :])
        p1psum.release()
        p1bpsum = tc.alloc_tile_pool(name="p1bps", bufs=1, space="PSUM")

        # fusedT [128, 10, 64]: chunks 0-4 = e.T ; 5-9 = memory.T broadcast
        fusedT_pse = p1bpsum.tile([128, 5, NTOK], bf16)
        for k in range(5):
            nc.tensor.transpose(out=fusedT_pse[:, k, :],
                                in_=e_sb[:, 128 * k:128 * (k + 1)],
                                identity=identb[0:64, 0:64])
        fusedT_psm = p1bpsum.tile([128, 5, NTOK], f32)
        for m in range(5):
            nc.tensor.matmul(out=fusedT_psm[:, m, :],
                             lhsT=mem_sb[:, 128 * m:128 * (m + 1)],
                             rhs=bsel_sb[:, :], start=True, stop=True)
        fusedT_sb = spool.tile([128, 10, NTOK], bf16)
        nc.scalar.copy(fusedT_sb[:, 0:5, :], fusedT_pse[:, :, :])
        nc.scalar.copy(fusedT_sb[:, 5:10, :], fusedT_psm[:, :, :])

        # ---------------- attention (batched over all tokens) ----------------
        attnw_sb = dmapool.tile([128, 10, HID], bf16)
        nc.sync.dma_start(out=attnw_sb[:, :, :],
                          in_=attnw_d[:, :, :].rearrange("k p o -> p k o"))
        attnb_sb = dmapool.tile([1, HID], bf16)
        nc.sync.dma_start(out=attnb_sb[:, :], in_=attnb_d[:, :])

        attn_ps = p1bpsum.tile([NTOK, HID], f32)
        for n in range(2):
            sl = slice(512 * n, min(HID, 512 * (n + 1)))
            for k in range(10):
                mm(out=attn_ps[:, sl], lhsT=fusedT_sb[:, k, :],
                   rhs=attnw_sb[:, k, sl], start=(k == 0), stop=False)
            mm(out=attn_ps[:, sl], lhsT=ones64[:, :],
               rhs=attnb_sb[:, sl], start=False, stop=True)
        # softmax over free dim, then context = softmax * memory
        nmx_sb = scpool.tile([NTOK, 1], f32)
        nc.vector.reduce_max(out=nmx_sb[:, :], in_=attn_ps[:, :], axis=AX.X,
                             negate=True)
        ex_sb = scpool.tile([NTOK, HID], f32)
        ssum_sb = scpool.tile([NTOK, 1], f32)
        nc.scalar.activation(ex_sb[:, :], attn_ps[:, :], AF.Exp,
                             bias=nmx_sb[:, 0:1], accum_out=ssum_sb[:, 0:1])
        rcp_sb = scpool.tile([NTOK, 1], f32)
        nc.vector.reciprocal(rcp_sb[:, :], ssum_sb[:, :])
        ctx_sb = scpool.tile([NTOK, HID], bf16)
        nc.vector.tensor_scalar_mul(ctx_sb[:, :], ex_sb[:, :], rcp_sb[:, 0:1])
        nc.vector.tensor_tensor(out=ctx_sb[:, :], in0=ctx_sb[:, :], in1=mexp_sb[:, :],
                                op=ALU.mult)
        ctxT_ps = p1bpsum.tile([128, 5, NTOK], bf16)
        for k in range(5):
            nc.tensor.transpose(out=ctxT_ps[:, k, :],
                                in_=ctx_sb[:, 128 * k:128 * (k + 1)],
                                identity=identb[0:64, 0:64])
        ctxT_sb = spool.tile([128, 5, NTOK], bf16)
        nc.scalar.copy(ctxT_sb[:, :, :], ctxT_ps[:, :, :])
        p1bpsum.release()
        scpool.release()

        p2psum = tc.alloc_tile_pool(name="p2ps", bufs=1, space="PSUM")
        PT_ps = p2psum.tile([128, NG, NTOK], f32)
        for m in range(NG):
            for k in range(10):
                rhsT = fusedT_sb[:, k, :] if k < 5 else ctxT_sb[:, k - 5, :]
                mm(out=PT_ps[:, m, :], lhsT=wih_sb[:, m, k, :],
                   rhs=rhsT, start=(k == 0), stop=False)
            mm(out=PT_ps[:, m, :], lhsT=bgate_sb[:, 128 * m:128 * (m + 1)],
               rhs=ones64[:, :], start=False, stop=True)
        PT_sb = spool.tile([128, NG, NTOK], bf16)
        nc.scalar.copy(PT_sb[:, :, :], PT_ps[:, :, :])
        p2psum.release()
        wihpool.release()
        nc.leave_named_scope("enc_attn", _sc_ea, False)
        mark("enc_attn")

        if upto == "pre":
            raise _PhaseExit(tc)
        _sc_ls = nc.enter_named_scope("lstm", False)[0]
        # ---------------- LSTM recurrence (transposed: gates.T on partitions) ----------------
        outsT_sb = spool.tile([128, 5, NTOK], bf16)   # h.T for every step
        c_sb = spool.tile([128, 5, BL], f32)

        # FC weight stream: allocate + DMA before the LSTM so transfers overlap it
        fcwpool = tc.alloc_tile_pool(name="fcw", bufs=1)
        lpsum = tc.alloc_tile_pool(name="lstm_ps", bufs=2, space="PSUM")
        lsp = tc.alloc_tile_pool(name="lstm_sb", bufs=2)
        CH = 1000
        fws = []
        for j in range(VOCAB // CH):
            fw = fcwpool.tile([128, 5, CH], bf16, tag="fw", bufs=9)
            nc.sync.dma_start(out=fw[:, :, :],
                              in_=fcw_d[:, :, CH * j:CH * (j + 1)].rearrange(
                                  "k p o -> p k o"))
            fcb_sb = fcwpool.tile([1, CH], bf16, tag="fcb", bufs=2)
            nc.sync.dma_start(out=fcb_sb[:, :], in_=fcb_d[:, CH * j:CH * (j + 1)])
            fws.append((fw, fcb_sb))
        for t in range(T):
            tt = slice(BL * t, BL * (t + 1))
            if t == 0:
                gsum = PT_sb
                gt = tt
            else:
                gt = slice(0, BL)
                gatesT_ps = lpsum.tile([128, NG, BL], f32, tag="gates")
                for m in range(NG):
                    for k in range(5):
                        mm(out=gatesT_ps[:, m, :],
                           lhsT=whh_sb[:, k, 128 * m:128 * (m + 1)],
                           rhs=outsT_sb[:, k, BL * (t - 1):BL * t],
                           start=(k == 0), stop=(k == 4))
                gsum = lsp.tile([128, NG, BL], f32, tag="gsum")
                nc.vector.tensor_tensor(out=gsum[:, :, :], in0=gatesT_ps[:, :, :],
                                        in1=PT_sb[:, :, tt], op=ALU.add)
            # nonlinearities: [i,f,o] sigmoid, [g] tanh (host permuted gate order)
            sig = lsp.tile([128, 15, BL], f32, tag="sig")
            tg = lsp.tile([128, 5, BL], f32, tag="tg")
            nc.scalar.activation(sig[:, :, :], gsum[:, 0:15, gt], AF.Sigmoid)
            nc.scalar.activation(tg[:, :, :], gsum[:, 15:20, gt], AF.Tanh)
            ig = lsp.tile([128, 5, BL], f32, tag="ig")
            nc.vector.tensor_tensor(out=ig[:, :, :], in0=sig[:, 0:5, :],
                                    in1=tg[:, :, :], op=ALU.mult)
            if t > 0:
                nc.vector.tensor_tensor(out=c_sb[:, :, :], in0=sig[:, 5:10, :],
                                        in1=c_sb[:, :, :], op=ALU.mult)
                nc.vector.tensor_tensor(out=c_sb[:, :, :], in0=c_sb[:, :, :],
                                        in1=ig[:, :, :], op=ALU.add)
            else:
                nc.vector.tensor_copy(out=c_sb[:, :, :], in_=ig[:, :, :])
            thc = lsp.tile([128, 5, BL], f32, tag="thc")
            nc.scalar.activation(thc[:, :, :], c_sb[:, :, :], AF.Tanh)
            nc.vector.tensor_tensor(out=outsT_sb[:, :, tt], in0=sig[:, 10:15, :],
                                    in1=thc[:, :, :], op=ALU.mult)
        lsp.release()
        lpsum.release()
        nc.leave_named_scope("lstm", _sc_ls, False)
        mark("lstm")

        if upto == "lstm":
            raise _PhaseExit(tc)
        _sc_fc = nc.enter_named_scope("fc", False)[0]
        # -------- FC to vocab (transposed): logits.T = fc_w @ outs.T + fc_b --------
        # vocab rows on partitions (M=125), tokens streaming (N=64); PSUM -> DRAM.
        fpsum = tc.alloc_tile_pool(name="fc_ps", bufs=2, space="PSUM")
        NCHK = CH // VC   # 8 vocab chunks per CH group = one full PSUM bank
        for j in range(VOCAB // CH):
            fw, fcb_sb = fws[j]
            ps = fpsum.tile([VC, NCHK, NTOK], f32, tag="ps")
            for s in range(NCHK):
                for k in range(5):
                    mm(out=ps[:, s, :], lhsT=fw[:, k, VC * s:VC * (s + 1)],
                       rhs=outsT_sb[:, k, :],
                       start=(k == 0), stop=False)
                mm(out=ps[:, s, :], lhsT=fcb_sb[:, VC * s:VC * (s + 1)],
                   rhs=ones64[:, :], start=False, stop=True)
            lo = spool.tile([VC, NCHK, NTOK], f32, tag="lo", bufs=2)
            nc.scalar.copy(lo[:, :, :], ps[:, :, :])
            nc.sync.dma_start(out=logits_d[j, :, :, :], in_=lo[:, :, :])
        fpsum.release()
        fcwpool.release()
        nc.leave_named_scope("fc", _sc_fc, False)
        mark("fc")
        spool.release()
        dmapool.release()
        cpool.release()
    except _PhaseExit:
        pass

    nc.finalize()
    return nc


def _prep_shared(inputs):
    """Host-side weight layout prep (shared across cores)."""
    import ml_dtypes
    bf = ml_dtypes.bfloat16
    f = np.float32
    perm = _gate_perm()
    w1 = inputs["cw1"].astype(f)
    w1b = w1.transpose(2, 3, 1, 0).reshape(27, 64)
    # block-diagonal stacked-images conv1 weight + bias row (K=55)
    w1s = np.zeros((55, 128), f)
    w1s[0:27, 0:64] = w1b
    w1s[27:54, 64:128] = w1b
    w1s[54, 0:64] = inputs["cb1"].astype(f)
    w1s[54, 64:128] = inputs["cb1"].astype(f)
    cb2t = inputs["cb2"].astype(f).reshape(128, 1).copy()
    w2t9 = inputs["cw2"].astype(f).transpose(2, 3, 1, 0).reshape(9, 64, 128)
    # tap pairs per image: im0 = [ky0; ky1] (dup half holds row+1), im1 = [ky1; ky0]
    w2p0 = np.concatenate([w2t9[0:3], w2t9[3:6]], axis=1)   # [3, 128, 128]
    w2p1 = np.concatenate([w2t9[3:6], w2t9[0:3]], axis=1)
    w2p = np.stack([w2p0, w2p1]).transpose(0, 2, 1, 3).copy()  # [2, 128, 3, 128]
    # singles (ky=2) duplicated in both partition halves
    w2s = np.concatenate([w2t9[6:9], w2t9[6:9]], axis=1).transpose(1, 0, 2).copy()
    w3t9 = inputs["cw3"].astype(f).transpose(2, 3, 1, 0).reshape(9, 128, 256)
    # [ky*kx=9, k2=2, 128, 512] -> [p=128, m=4, k2=2, tap=9, o=128]
    w4t9 = (inputs["cw4"].astype(f).transpose(2, 3, 1, 0).reshape(9, 2, 128, 4, 128)
            .transpose(2, 3, 1, 0, 4).copy())
    cb3t = inputs["cb3"].astype(f).reshape(2, 128).T.copy()
    cb4t = inputs["cb4"].astype(f).reshape(4, 128).T.copy()
    encwt = inputs["enc_w"].astype(f).T.reshape(4, 128, HID).copy()
    encbt = inputs["enc_b"].astype(f).reshape(5, 128).T.copy()
    attnwt = inputs["attn_w"].astype(f).T.reshape(10, 128, HID).copy()
    attnb = inputs["attn_b"].astype(f)[None, :]
    import concourse.mybir as _mb
    f8np = _mb.dt.np(_mb.dt.float8e4)
    wih = inputs["w_ih"].astype(f)[perm]
    whh = inputs["w_hh"].astype(f)[perm]
    # wih.T [2H, 4H] -> [m=20, p=128(k), kk=10, g=128]
    wiht = wih.T.reshape(10, 128, 20, 128).transpose(2, 1, 0, 3).copy()
    whht = whh.T.reshape(5, 128, 4 * HID).copy()
    bgate = (inputs["b_ih"].astype(f) + inputs["b_hh"].astype(f))[perm][None, :]
    fcwt = inputs["fc_w"].astype(f).T.reshape(5, 128, VOCAB).copy()
    fcb = inputs["fc_b"].astype(f)[None, :]
    bsel = np.zeros((BL, NTOK), f)
    for p in range(NTOK):
        bsel[p % BL, p] = 1.0
    return dict(w1s=w1s.astype(bf), cb2t=cb2t,
                w2p=w2p.astype(bf), w2s=w2s.astype(bf),
                w3t9=w3t9.astype(bf), w4t9=w4t9.astype(bf),
                cb3t=cb3t, cb4t=cb4t, encwt=encwt, encbt=encbt,
                attnwt=attnwt.astype(bf), attnb=attnb.astype(bf),
                wiht=wiht.astype(bf), whht=whht.astype(bf), bgate=bgate.astype(bf),
                fcwt=fcwt.astype(bf), fcb=fcb.astype(bf), bsel=bsel,
                emb=inputs["emb"].astype(f).astype(bf))


def _make_in_maps(inputs):
    shared = _prep_shared(inputs)
    images = np.asarray(inputs["images"], np.float32)
    captions = np.asarray(inputs["captions"])

    import ml_dtypes
    imgp = np.zeros((16, 3, 226, 226), np.float32)
    imgp[:, :, 1:225, 1:225] = images
    s = imgp.strides
    win = np.lib.stride_tricks.as_strided(
        imgp, shape=(16, 3, 3, 3, 224, 224),
        strides=(s[0], s[1], s[2], s[3], s[2], s[3]))
    # rows (ky, kx, c) to match w1 layout
    imcol = win.transpose(0, 2, 3, 1, 4, 5).reshape(16, 27, 224 * 224)
    imgp = imcol.astype(ml_dtypes.bfloat16)
    ones_row = np.ones((1, 224 * 224), ml_dtypes.bfloat16)
    in_maps = []
    for c in range(NCORES):
        caps = captions[BL * c:BL * (c + 1)].astype(np.int64).T.reshape(NTOK, 1)
        m = dict(shared)
        m["img"] = np.concatenate(
            [imgp[BL * c], imgp[BL * c + 1], ones_row], axis=0)
        m["caps"] = caps.astype(np.int32)
        in_maps.append(m)
    return in_maps


def kernel(**inputs):
    from concourse.bass_utils import run_bass_kernel_spmd

    if "nc" not in _NC_CACHE:
        _NC_CACHE["nc"] = build_bass()
    nc = _NC_CACHE["nc"]

    in_maps = _make_in_maps(inputs)
    res = run_bass_kernel_spmd(nc, in_maps, list(range(NCORES)))
    # logits come back as [j=10, v=125, s=8, tok]: vocab index = j*1000+s*125+v
    out = np.concatenate(
        [res.results[c]["logits"].transpose(0, 2, 1, 3)
             .reshape(VOCAB, T, BL).transpose(2, 1, 0)
         for c in range(NCORES)], axis=0)
    return out



# revision 75
# speedup vs baseline: 1.0952x; 1.0952x over previous
"""Trainium2 Bass kernel for CNN-encoder + attention-LSTM captioner + vocab FC.

Sharding: pure data-parallel over batch (16 images -> 8 cores x 2 images).
All weights replicated; no collectives. Host slices inputs / concatenates
outputs (logits come back vocab-major and are untransposed on the host).

Design notes (per core, BL=2 images, T=32 steps, all compute bf16/f32):
  - conv1 runs ONCE for both images: channels of im0 sit in partitions 0-63
    and im1 in 64-127 via a block-diagonal [55,128] weight (27 im2col rows per
    image + a ones row that folds in the bias). Pooling is
    relu(maxpool) == max(psum_even, relu(psum_odd)) rowmaxed into a staging
    tile; SBUF->SBUF DMAs then assemble per-image "tap pair" layouts
    [ch | ch shifted one row] so conv2 contracts K=128 for 6 of 9 taps.
  - conv2/3/4 are shift-accumulate 3x3 convs at the PE roofline; conv4's
    spatial mean rides the activation's accum_out.
  - the whole attention/gates precompute is batched over all 64 tokens and
    kept TRANSPOSED (gates on partitions): P.T = w_ih @ [e;ctx].T + b.
  - each LSTM step seeds PSUM with P_t via an identity matmul, accumulates
    W_hh @ h.T (weights stationary, h streaming N=2), and runs all gate
    nonlinearities/elementwise on [128, 5..20, 2] tiles so every engine lane
    is used; h.T lands directly in the FC-ready outsT buffer (no transposes).
  - FC computes logits.T (vocab rows on partitions, M=125) with fc_w
    prefetched during the LSTM; output DRAM layout is DMA-friendly
    [10, 125, 8, 64] and untransposed on the host.
  - big weights stream behind the image DMAs; w_ih prefetches into a scoped
    pool that is released before the fc_w stream needs the SBUF.
"""

import os
import numpy as np

os.environ.setdefault("MYCRO_LOCAL_CACHE", "1")

HID = 640
VOCAB = 10000
T = 32
BL = 2            # local batch per core
NTOK = T * BL     # 64
NCORES = 8

F32 = None  # set lazily (mybir.dt.float32)


class _PhaseExit(Exception):
    def __init__(self, tc):
        self.tc = tc

_NC_CACHE = {}
PHASE_MARKS = []   # (phase_name, inst_count_at_phase_end) recorded during build


def _gate_perm():
    # reference gate order [i, f, g, o] -> kernel order [i, f, o, g]
    return np.concatenate([
        np.arange(0, 1280),          # i, f
        np.arange(1920, 2560),       # o
        np.arange(1280, 1920),       # g
    ])


def build_bass(upto=None):
    import os
    upto = upto or os.environ.get("KERNEL_UPTO", "all")
    import concourse.bass as bass
    from concourse import bacc
    import concourse.tile_sem_assignment as tsa
    # Cap HWDGE sem lanes so pool-transition fan-ins stay under the
    # per-instruction sync-wait slot limits in walrus codegen.
    tsa.NUM_HWDGE_SEMS = 4
    import concourse.mybir as mybir
    import concourse.tile as tile
    from concourse.masks import make_identity

    f32 = mybir.dt.float32
    i32 = mybir.dt.int32
    AF = mybir.ActivationFunctionType
    ALU = mybir.AluOpType
    AX = mybir.AxisListType

    nc = bacc.Bacc(None)
    bf16 = mybir.dt.bfloat16
    f8 = mybir.dt.float8e4

    PHASE_MARKS.clear()

    def mark(name):
        PHASE_MARKS.append((name, len(nc.inst_map)))

    def mm(out, lhsT, rhs, **kw):
        nc.tensor.matmul(out=out, lhsT=lhsT, rhs=rhs, **kw)

    # ---------------- DRAM parameters ----------------
    img_d = nc.declare_dram_parameter("img", [55, 224 * 224], bf16, isOutput=False)
    caps_d = nc.declare_dram_parameter("caps", [NTOK, 1], i32, isOutput=False)
    w1s_d = nc.declare_dram_parameter("w1s", [55, 128], bf16, isOutput=False)
    cb2_d = nc.declare_dram_parameter("cb2t", [128, 1], f32, isOutput=False)
    w2p_d = nc.declare_dram_parameter("w2p", [2, 128, 3, 128], bf16, isOutput=False)
    w2s_d = nc.declare_dram_parameter("w2s", [128, 3, 128], bf16, isOutput=False)
    w3t9_d = nc.declare_dram_parameter("w3t9", [9, 128, 256], bf16, isOutput=False)
    w4t9_d = nc.declare_dram_parameter("w4t9", [128, 4, 2, 9, 128], bf16, isOutput=False)
    cb3_d = nc.declare_dram_parameter("cb3t", [128, 2], f32, isOutput=False)
    cb4_d = nc.declare_dram_parameter("cb4t", [128, 4], f32, isOutput=False)
    encw_d = nc.declare_dram_parameter("encwt", [4, 128, HID], f32, isOutput=False)
    encb_d = nc.declare_dram_parameter("encbt", [128, 5], f32, isOutput=False)
    emb_d = nc.declare_dram_parameter("emb", [VOCAB, HID], bf16, isOutput=False)
    attnw_d = nc.declare_dram_parameter("attnwt", [10, 128, HID], bf16, isOutput=False)
    attnb_d = nc.declare_dram_parameter("attnb", [1, HID], bf16, isOutput=False)
    wih_d = nc.declare_dram_parameter("wiht", [20, 128, 10, 128], bf16, isOutput=False)
    whh_d = nc.declare_dram_parameter("whht", [5, 128, 4 * HID], bf16, isOutput=False)
    bgate_d = nc.declare_dram_parameter("bgate", [1, 4 * HID], bf16, isOutput=False)
    fcw_d = nc.declare_dram_parameter("fcwt", [5, 128, VOCAB], bf16, isOutput=False)
    fcb_d = nc.declare_dram_parameter("fcb", [1, VOCAB], bf16, isOutput=False)
    bsel_d = nc.declare_dram_parameter("bsel", [BL, NTOK], f32, isOutput=False)
    # logits stored transposed + group-blocked: [group, vocab_row, chunk, tok]
    NVC = 80          # vocab chunks
    VC = VOCAB // NVC  # 125 vocab rows per chunk
    logits_d = nc.declare_dram_parameter("logits", [10, VC, 8, NTOK], f32,
                                         isOutput=True)

    try:
      with tile.TileContext(nc) as tc:
        # ---------------- persistent constants ----------------
        cpool = tc.alloc_tile_pool(name="const", bufs=1)
        # pool for all DMA-written tiles: never released mid-kernel so that
        # SBUF zone reuse never makes compute ops wait on DMA queue sems
        dmapool = tc.alloc_tile_pool(name="dmat", bufs=1)
        ident = cpool.tile([128, 128], f32)
        make_identity(nc, ident[:, :])
        identb = cpool.tile([128, 128], bf16)
        make_identity(nc, identb[:, :])
        ones64 = cpool.tile([1, 64], bf16)
        nc.gpsimd.memset(ones64[:, :], 1.0)
        bsel_sb = dmapool.tile([BL, NTOK], f32)
        nc.sync.dma_start(out=bsel_sb[:, :], in_=bsel_d[:, :])
        feat_sb = cpool.tile([128, 4, BL], f32)   # feat.T, K-chunked [128,4] per img

        w1s_sb = dmapool.tile([55, 128], bf16)
        nc.sync.dma_start(out=w1s_sb[:, :], in_=w1s_d[:, :])
        cb2_sb = dmapool.tile([128, 1], f32)
        nc.sync.dma_start(out=cb2_sb[:, :], in_=cb2_d[:, :])
        w2p_sb = dmapool.tile([128, 2, 3, 128], bf16)
        nc.sync.dma_start(out=w2p_sb[:, :, :, :],
                          in_=w2p_d[:, :, :, :].rearrange("i p t o -> p i t o"))
        w2s_sb = dmapool.tile([128, 3, 128], bf16)
        nc.sync.dma_start(out=w2s_sb[:, :, :], in_=w2s_d[:, :, :])
        w3_sb = dmapool.tile([128, 9, 256], bf16)
        nc.sync.dma_start(out=w3_sb[:, :, :], in_=w3t9_d[:, :, :].rearrange("t p o -> p t o"))
        cb3_sb = dmapool.tile([128, 2], f32)
        nc.sync.dma_start(out=cb3_sb[:, :], in_=cb3_d[:, :])
        cb4_sb = dmapool.tile([128, 4], f32)
        nc.sync.dma_start(out=cb4_sb[:, :], in_=cb4_d[:, :])

        # ---------------- conv tower ----------------
        # per-image padded intermediates; only borders need zeroing (interiors
        # are fully rewritten). x2 tiles live in their own pool so their SBUF
        # frees right after conv2(im1), making room to prefetch w_ih early.
        spool = tc.alloc_tile_pool(name="seq", bufs=1)
        padpool = tc.alloc_tile_pool(name="pads", bufs=1)
        x2pool = tc.alloc_tile_pool(name="x2p", bufs=1)
        # staging: [im0ch | im1ch] padded pool1 output + an extra always-zero
        # row 114 so the row+1-shifted dup copies need no edge memset
        x2s = x2pool.tile([128, 115, 114], bf16)
        nc.vector.memset(x2s[:, 0, :], 0.0)
        nc.vector.memset(x2s[:, 113:115, :], 0.0)
        nc.vector.memset(x2s[:, :, 0], 0.0)
        nc.vector.memset(x2s[:, :, 113], 0.0)
        x2im = []
        for im in range(BL):
            x2t = x2pool.tile([128, 114, 114], bf16)
            x2im.append(x2t)
        x3im = [None, None]
        x4im = [None, None]

        # ---- conv1 (3->64), both images stacked on partitions (M=64ch x 2im),
        # im2col K=54 + ones-row bias; pool chain: DVE xmax (psum evict),
        # Pool rowmax + relu ----
        _sc1 = nc.enter_named_scope("conv1", False)[0]
        c1pool = tc.alloc_tile_pool(name="c1", bufs=4)
        c1psum = tc.alloc_tile_pool(name="c1p", bufs=4, space="PSUM")
        R = 16
        for ch in range(224 // R):
            Y = R * ch
            rh = dmapool.tile([55, R * 224], bf16, tag="rh", bufs=2)
            nc.sync.dma_start(out=rh[:, :],
                              in_=img_d[:, Y * 224:(Y + R) * 224])
            rhv = rh.rearrange("p (r x) -> p r x", x=224)
            for q in range(4):
                ps = c1psum.tile([128, 2, 448], f32, padded_shape=[128, 2, 512],
                                 tag="ps")
                for s in range(2):
                    r0 = q * 4 + s * 2
                    mm(out=ps[:, s, :], lhsT=w1s_sb[:, :],
                       rhs=rhv[:, r0:r0 + 2, :], start=True, stop=True)
                # relu(maxpool) == max(even, relu(odd), rows): ACT relu-evicts
                # only the odd columns; DVE maxes them against the raw PSUM
                # evens (xm >= 0 always), then row-pools into x2s.
                a1 = c1pool.tile([128, 2, 224], bf16, tag="a1")
                nc.scalar.activation(a1[:, :, :], ps[:, :, 1:448:2], AF.Relu)
                xm = c1pool.tile([128, 2, 224], bf16, tag="xm")
                nc.vector.tensor_tensor(out=xm[:, :, :],
                                        in0=ps[:, :, 0:448:2],
                                        in1=a1[:, :, :], op=ALU.max)
                oy = (R * ch + 4 * q) // 2
                nc.vector.tensor_tensor(out=x2s[:, oy + 1:oy + 3, 1:113],
                                        in0=xm[:, :, 0:112],
                                        in1=xm[:, :, 112:224], op=ALU.max)
        w4_sb = dmapool.tile([128, 4, 2, 9, 128], bf16)
        nc.sync.dma_start(out=w4_sb[:, :, :, :, :], in_=w4t9_d[:, :, :, :, :])
        encw_sb = dmapool.tile([128, 4, HID], f32)
        nc.sync.dma_start(out=encw_sb[:, :, :], in_=encw_d[:, :, :].rearrange("k p o -> p k o"))
        encb_sb = dmapool.tile([128, 5], f32)
        nc.sync.dma_start(out=encb_sb[:, :], in_=encb_d[:, :])
        attnw_sb = dmapool.tile([128, 10, HID], bf16)
        nc.sync.dma_start(out=attnw_sb[:, :, :],
                          in_=attnw_d[:, :, :].rearrange("k p o -> p k o"))
        attnb_sb = dmapool.tile([1, HID], bf16)
        nc.sync.dma_start(out=attnb_sb[:, :], in_=attnb_d[:, :])
        whh_sb = dmapool.tile([128, 5, 4 * HID], bf16)
        nc.sync.dma_start(out=whh_sb[:, :, :],
                          in_=whh_d[:, :, :].rearrange("k p o -> p k o"))
        bgate_sb = dmapool.tile([1, 4 * HID], bf16)
        nc.sync.dma_start(out=bgate_sb[:, :], in_=bgate_d[:, :])
        idx_sb = dmapool.tile([NTOK, 1], i32)
        nc.sync.dma_start(out=idx_sb[:, :], in_=caps_d[:, :])
        e_sb = dmapool.tile([NTOK, HID], bf16)
        nc.gpsimd.indirect_dma_start(
            out=e_sb[:, :], out_offset=None,
            in_=emb_d[:, :],
            in_offset=bass.IndirectOffsetOnAxis(ap=idx_sb[:, :1], axis=0),
        )
        c1psum.release()
        c1pool.release()
        # assemble per-image tap-pair layouts from the staging tile:
        # x2im0 = [im0 | im0 shifted+1], x2im1 = [im1 shifted+1 | im1]
        for r0, r1 in ((0, 57), (57, 114)):
            nc.sync.dma_start(out=x2im[0][0:64, r0:r1, :],
                              in_=x2s[0:64, r0:r1, :])
            nc.sync.dma_start(out=x2im[0][64:128, r0:r1, :],
                              in_=x2s[0:64, r0 + 1:r1 + 1, :])
            nc.sync.dma_start(out=x2im[1][64:128, r0:r1, :],
                              in_=x2s[64:128, r0:r1, :])
            nc.sync.dma_start(out=x2im[1][0:64, r0:r1, :],
                              in_=x2s[64:128, r0 + 1:r1 + 1, :])
        nc.leave_named_scope("conv1", _sc1, False)
        mark("conv1")
        NG = 4 * HID // 128   # 20 gate chunks: i=0-4, f=5-9, o=10-14, g=15-19
        wih_sb = None
        for im in range(BL):
            _sc = nc.enter_named_scope(f"conv_im{im}", False)[0]

            # ---- conv2 (64->128): 3 K=128 tap-pairs + 3 K=64 singles ----
            x3t = padpool.tile([128, 58, 58], bf16)
            nc.vector.memset(x3t[:, :, :], 0.0)
            x3im[im] = x3t
            x4t = padpool.tile([128, 2, 30, 30], bf16)
            nc.vector.memset(x4t[:, :, :, :], 0.0)
            x4im[im] = x4t
            c2psum = tc.alloc_tile_pool(name=f"c2p_{im}", bufs=3, space="PSUM")
            c2pool = tc.alloc_tile_pool(name=f"c2_{im}", bufs=2)
            sb = 0 if im == 0 else 64   # singles partition base for this image
            for tl in range(14):  # 8 output rows per tile
                ps = c2psum.tile([128, 2, 448], f32, padded_shape=[128, 2, 512], tag="ps")
                for s in range(2):
                    y0 = tl * 8 + s * 4
                    for kx in range(3):
                        mm(
                            out=ps[:, s, :], lhsT=w2p_sb[:, im, kx, :],
                            rhs=x2im[im][:, y0:y0 + 4, kx:kx + 112],
                            start=(kx == 0), stop=False,
                        )
                    for kx in range(3):
                        mm(
                            out=ps[:, s, :], lhsT=w2s_sb[sb:sb + 64, kx, :],
                            rhs=x2im[im][sb:sb + 64, y0 + 2:y0 + 6, kx:kx + 112],
                            start=False, stop=(kx == 2),
                        )
                a2 = c2pool.tile([128, 8, 112], bf16, tag="a2")
                nc.scalar.activation(
                    a2.rearrange("p (a y) x -> p a y x", a=2),
                    ps.rearrange("p a (y x) -> p a y x", x=112),
                    AF.Relu, bias=cb2_sb[:, 0:1])
                t2 = c2pool.tile([128, 8, 56], bf16, tag="t2")
                nc.vector.tensor_tensor(
                    out=t2[:, :, :], in0=a2[:, :, 0:112:2], in1=a2[:, :, 1:112:2],
                    op=ALU.max,
                )
                nc.vector.tensor_tensor(
                    out=x3im[im][:, tl * 4 + 1:tl * 4 + 5, 1:57],
                    in0=t2[:, 0:8:2, :], in1=t2[:, 1:8:2, :],
                    op=ALU.max,
                )
            c2psum.release()
            c2pool.release()
            if im == BL - 1:
                # x2 SBUF is dead: free it and prefetch the full w_ih under
                # the remaining conv3/conv4 compute
                x2pool.release()
                wihpool = tc.alloc_tile_pool(name="wihp", bufs=1)
                wih_sb = wihpool.tile([128, NG, 10, 128], bf16)
                for g in range(4):
                    nc.sync.dma_start(
                        out=wih_sb[:, 5 * g:5 * (g + 1), :, :],
                        in_=wih_d[5 * g:5 * (g + 1), :, :, :].rearrange(
                            "m p k o -> p m k o"))

            # ---- conv3 (128->256) K=128, bias via ACT evict, pool -> x4_pad ----
            c3psum = tc.alloc_tile_pool(name=f"c3p_{im}", bufs=3, space="PSUM")
            c3pool = tc.alloc_tile_pool(name=f"c3_{im}", bufs=2)
            for m in range(2):
                for tl in range(7):  # 8 output rows per tile
                    ps = c3psum.tile([128, 448], f32, padded_shape=[128, 512], tag="ps")
                    y0 = tl * 8
                    for ky in range(3):
                        for kx in range(3):
                            tap = ky * 3 + kx
                            rhs = x3im[im][:, y0 + ky:y0 + ky + 8, kx:kx + 56]
                            mm(
                                out=ps[:, :],
                                lhsT=w3_sb[:, tap, 128 * m:128 * (m + 1)],
                                rhs=rhs,
                                start=(tap == 0), stop=(tap == 8),
                            )
                    a3 = c3pool.tile([128, 8, 56], bf16, tag="a3")
                    nc.scalar.activation(
                        a3[:, :, :],
                        ps.rearrange("p (y x) -> p y x", x=56),
                        AF.Relu, bias=cb3_sb[:, m:m + 1])
                    t3 = c3pool.tile([128, 8, 28], bf16, tag="t3")
                    nc.vector.tensor_tensor(
                        out=t3[:, :, :], in0=a3[:, :, 0:56:2], in1=a3[:, :, 1:56:2],
                        op=ALU.max,
                    )
                    nc.vector.tensor_tensor(
                        out=x4im[im][:, m, tl * 4 + 1:tl * 4 + 5, 1:29],
                        in0=t3[:, 0:8:2, :], in1=t3[:, 1:8:2, :],
                        op=ALU.max,
                    )
            c3psum.release()
            c3pool.release()

            # ---- conv4 (256->512) K=256 (2 chunks), no pool; mean via accum_out ----
            ipool = tc.alloc_tile_pool(name=f"img{im}", bufs=1)
            c4psum = tc.alloc_tile_pool(name=f"c4p_{im}", bufs=3, space="PSUM")
            c4pool = tc.alloc_tile_pool(name=f"c4_{im}", bufs=2)
            msum = ipool.tile([128, 4, 2], f32)
            for m in range(4):
                w4m = w4_sb[:, m]
                ps = c4psum.tile([128, 2, 392], f32, padded_shape=[128, 2, 512], tag="ps")
                for s in range(2):
                    y0 = s * 14
                    first = True
                    for ky in range(3):
                        for kx in range(3):
                            tap = ky * 3 + kx
                            for k2 in range(2):
                                rhs = x4im[im][:, k2, y0 + ky:y0 + ky + 14, kx:kx + 28]
                                mm(
                                    out=ps[:, s, :],
                                    lhsT=w4m[:, k2, tap, :],
                                    rhs=rhs,
                                    start=first, stop=(tap == 8 and k2 == 1),
                                )
                                first = False
                a4 = c4pool.tile([128, 2, 392], bf16, tag="a4")
                for s in range(2):
                    nc.scalar.activation(a4[:, s, :], ps[:, s, :], AF.Relu,
                                         bias=cb4_sb[:, m:m + 1],
                                         accum_out=msum[:, m, s:s + 1])
            c4psum.release()
            c4pool.release()
            # feat.T[:, m] = (msum[:,m,0] + msum[:,m,1]) / 784
            tmpf = ipool.tile([128, 4], f32)
            nc.vector.tensor_tensor(out=tmpf[:, :], in0=msum[:, :, 0], in1=msum[:, :, 1],
                                    op=ALU.add)
            nc.vector.tensor_scalar_mul(feat_sb[:, :, im], tmpf[:, :], 1.0 / 784.0)
            ipool.release()
            nc.leave_named_scope(f"conv_im{im}", _sc, False)
            mark(f"conv_im{im}")

        if upto == "conv":
            raise _PhaseExit(tc)

        # ---------------- encoder linear: memory.T = enc_w @ feat.T + enc_b ----------------
        _sc_ea = nc.enter_named_scope("enc_attn", False)[0]
        scpool = tc.alloc_tile_pool(name="scratch", bufs=1)
        p1psum = tc.alloc_tile_pool(name="p1ps", bufs=1, space="PSUM")
        memT_ps = p1psum.tile([128, 5, BL], f32)
        for m in range(5):
            for k in range(4):
                nc.tensor.matmul(
                    out=memT_ps[:, m, :],
                    lhsT=encw_sb[:, k, 128 * m:128 * (m + 1)],
                    rhs=feat_sb[:, k, :],
                    start=(k == 0), stop=(k == 3),
                )
        memT_sb = spool.tile([128, 5, BL], f32)
        for m in range(5):
            nc.vector.tensor_scalar_add(memT_sb[:, m, :], memT_ps[:, m, :],
                                        encb_sb[:, m:m + 1])
        # memory non-transposed [2, 640]
        mem_ps = p1psum.tile([BL, HID], f32)
        for m in range(5):
            nc.tensor.transpose(out=mem_ps[:, 128 * m:128 * (m + 1)],
                                in_=memT_sb[:, m, :], identity=ident[:, :])
        mem_sb = scpool.tile([BL, HID], f32)
        nc.scalar.copy(mem_sb[:, :], mem_ps[:, :])

        # memory broadcast to all tokens [64, 640] via bsel matmul
        mexp_ps = p1psum.tile([NTOK, HID], f32)
        for n in range(2):
            sl = slice(512 * n, min(HID, 512 * (n + 1)))
            nc.tensor.matmul(out=mexp_ps[:, sl], lhsT=bsel_sb[:, :], rhs=mem_sb[:, sl],
                             start=True, stop=True)
        mexp_sb = scpool.tile([NTOK, HID], f32)
        nc.scalar.copy(mexp_sb[:, :], mexp_ps[:, :])
        p1psum.release()
        p1bpsum = tc.alloc_tile_pool(name="p1bps", bufs=1, space="PSUM")

        # fusedT [128, 10, 64]: chunks 0-4 = e.T ; 5-9 = memory.T broadcast
        fusedT_pse = p1bpsum.tile([128, 5, NTOK], bf16)
        for k in range(5):
            nc.tensor.transpose(out=fusedT_pse[:, k, :],
                                in_=e_sb[:, 128 * k:128 * (k + 1)],
                                identity=identb[0:64, 0:64])
        fusedT_psm = p1bpsum.tile([128, 5, NTOK], f32)
        for m in range(5):
            nc.tensor.matmul(out=fusedT_psm[:, m, :],
                             lhsT=mem_sb[:, 128 * m:128 * (m + 1)],
                             rhs=bsel_sb[:, :], start=True, stop=True)
        fusedT_sb = spool.tile([128, 10, NTOK], bf16)
        nc.scalar.copy(fusedT_sb[:, 0:5, :], fusedT_pse[:, :, :])
        nc.scalar.copy(fusedT_sb[:, 5:10, :], fusedT_psm[:, :, :])

        # ---------------- attention (batched over all tokens) ----------------
        attnw_sb = dmapool.tile([128, 10, HID], bf16)
        nc.sync.dma_start(out=attnw_sb[:, :, :],
                          in_=attnw_d[:, :, :].rearrange("k p o -> p k o"))
        attnb_sb = dmapool.tile([1, HID], bf16)
        nc.sync.dma_start(out=attnb_sb[:, :], in_=attnb_d[:, :])

        attn_ps = p1bpsum.tile([NTOK, HID], f32)
        for n in range(2):
            sl = slice(512 * n, min(HID, 512 * (n + 1)))
            for k in range(10):
                mm(out=attn_ps[:, sl], lhsT=fusedT_sb[:, k, :],
                   rhs=attnw_sb[:, k, sl], start=(k == 0), stop=False)
            mm(out=attn_ps[:, sl], lhsT=ones64[:, :],
               rhs=attnb_sb[:, sl], start=False, stop=True)
        # softmax over free dim, then context = softmax * memory
        nmx_sb = scpool.tile([NTOK, 1], f32)
        nc.vector.reduce_max(out=nmx_sb[:, :], in_=attn_ps[:, :], axis=AX.X,
                             negate=True)
        ex_sb = scpool.tile([NTOK, HID], f32)
        ssum_sb = scpool.tile([NTOK, 1], f32)
        nc.scalar.activation(ex_sb[:, :], attn_ps[:, :], AF.Exp,
                             bias=nmx_sb[:, 0:1], accum_out=ssum_sb[:, 0:1])
        rcp_sb = scpool.tile([NTOK, 1], f32)
        nc.vector.reciprocal(rcp_sb[:, :], ssum_sb[:, :])
        ctx_sb = scpool.tile([NTOK, HID], bf16)
        nc.vector.tensor_scalar_mul(ctx_sb[:, :], ex_sb[:, :], rcp_sb[:, 0:1])
        nc.vector.tensor_tensor(out=ctx_sb[:, :], in0=ctx_sb[:, :], in1=mexp_sb[:, :],
                                op=ALU.mult)
        ctxT_ps = p1bpsum.tile([128, 5, NTOK], bf16)
        for k in range(5):
            nc.tensor.transpose(out=ctxT_ps[:, k, :],
                                in_=ctx_sb[:, 128 * k:128 * (k + 1)],
                                identity=identb[0:64, 0:64])
        ctxT_sb = spool.tile([128, 5, NTOK], bf16)
        nc.scalar.copy(ctxT_sb[:, :, :], ctxT_ps[:, :, :])
        p1bpsum.release()
        scpool.release()

        p2psum = tc.alloc_tile_pool(name="p2ps", bufs=1, space="PSUM")
        PT_ps = p2psum.tile([128, NG, NTOK], f32)
        for m in range(NG):
            for k in range(10):
                rhsT = fusedT_sb[:, k, :] if k < 5 else ctxT_sb[:, k - 5, :]
                mm(out=PT_ps[:, m, :], lhsT=wih_sb[:, m, k, :],
                   rhs=rhsT, start=(k == 0), stop=False)
            mm(out=PT_ps[:, m, :], lhsT=bgate_sb[:, 128 * m:128 * (m + 1)],
               rhs=ones64[:, :], start=False, stop=True)
        PT_sb = spool.tile([128, NG, NTOK], bf16)
        nc.scalar.copy(PT_sb[:, :, :], PT_ps[:, :, :])
        p2psum.release()
        wihpool.release()
        padpool.release()
        nc.leave_named_scope("enc_attn", _sc_ea, False)
        mark("enc_attn")

        if upto == "pre":
            raise _PhaseExit(tc)
        _sc_ls = nc.enter_named_scope("lstm", False)[0]
        # ---------------- LSTM recurrence (transposed: gates.T on partitions) ----------------
        outsT_sb = spool.tile([128, 5, NTOK], bf16)   # h.T for every step
        c_sb = spool.tile([128, 5, BL], f32)

        # FC weight stream: allocate + DMA before the LSTM so transfers overlap it
        fcwpool = tc.alloc_tile_pool(name="fcw", bufs=1)
        lpsum = tc.alloc_tile_pool(name="lstm_ps", bufs=2, space="PSUM")
        lsp = tc.alloc_tile_pool(name="lstm_sb", bufs=2)
        CH = 1000
        fws = []
        for j in range(VOCAB // CH):
            fw = fcwpool.tile([128, 5, CH], bf16, tag="fw", bufs=10)
            nc.sync.dma_start(out=fw[:, :, :],
                              in_=fcw_d[:, :, CH * j:CH * (j + 1)].rearrange(
                                  "k p o -> p k o"))
            fcb_sb = fcwpool.tile([1, CH], bf16, tag="fcb", bufs=2)
            nc.sync.dma_start(out=fcb_sb[:, :], in_=fcb_d[:, CH * j:CH * (j + 1)])
            fws.append((fw, fcb_sb))
        for t in range(T):
            tt = slice(BL * t, BL * (t + 1))
            if t == 0:
                gsum = PT_sb
                gt = tt
            else:
                gt = slice(0, BL)
                gatesT_ps = lpsum.tile([128, NG, BL], f32, tag="gates")
                for m in range(NG):
                    # seed the accumulator with P_t via identity matmul, then
                    # accumulate W_hh @ h
                    mm(out=gatesT_ps[:, m, :], lhsT=identb[:, :],
                       rhs=PT_sb[:, m, tt], start=True, stop=False)
                    for k in range(5):
                        mm(out=gatesT_ps[:, m, :],
                           lhsT=whh_sb[:, k, 128 * m:128 * (m + 1)],
                           rhs=outsT_sb[:, k, BL * (t - 1):BL * t],
                           start=False, stop=(k == 4))
                gsum = gatesT_ps
            # nonlinearities: [i,f,o] sigmoid, [g] tanh (host permuted gate order)
            sig = lsp.tile([128, 15, BL], f32, tag="sig")
            tg = lsp.tile([128, 5, BL], f32, tag="tg")
            nc.scalar.activation(sig[:, :, :], gsum[:, 0:15, gt], AF.Sigmoid)
            nc.scalar.activation(tg[:, :, :], gsum[:, 15:20, gt], AF.Tanh)
            # f*c first: it only needs sig, so DVE overlaps ACT's tanh
            if t > 0:
                nc.vector.tensor_tensor(out=c_sb[:, :, :], in0=sig[:, 5:10, :],
                                        in1=c_sb[:, :, :], op=ALU.mult)
            ig = lsp.tile([128, 5, BL], f32, tag="ig")
            nc.vector.tensor_tensor(out=ig[:, :, :], in0=sig[:, 0:5, :],
                                    in1=tg[:, :, :], op=ALU.mult)
            if t > 0:
                nc.vector.tensor_tensor(out=c_sb[:, :, :], in0=c_sb[:, :, :],
                                        in1=ig[:, :, :], op=ALU.add)
            else:
                nc.vector.tensor_copy(out=c_sb[:, :, :], in_=ig[:, :, :])
            thc = lsp.tile([128, 5, BL], f32, tag="thc")
            nc.scalar.activation(thc[:, :, :], c_sb[:, :, :], AF.Tanh)
            nc.vector.tensor_tensor(out=outsT_sb[:, :, tt], in0=sig[:, 10:15, :],
                                    in1=thc[:, :, :], op=ALU.mult)
        lsp.release()
        lpsum.release()
        nc.leave_named_scope("lstm", _sc_ls, False)
        mark("lstm")

        if upto == "lstm":
            raise _PhaseExit(tc)
        _sc_fc = nc.enter_named_scope("fc", False)[0]
        # -------- FC to vocab (transposed): logits.T = fc_w @ outs.T + fc_b --------
        # vocab rows on partitions (M=125), tokens streaming (N=64); PSUM -> DRAM.
        fpsum = tc.alloc_tile_pool(name="fc_ps", bufs=2, space="PSUM")
        NCHK = CH // VC   # 8 vocab chunks per CH group = one full PSUM bank
        for j in range(VOCAB // CH):
            fw, fcb_sb = fws[j]
            ps = fpsum.tile([VC, NCHK, NTOK], f32, tag="ps")
            for s in range(NCHK):
                for k in range(5):
                    mm(out=ps[:, s, :], lhsT=fw[:, k, VC * s:VC * (s + 1)],
                       rhs=outsT_sb[:, k, :],
                       start=(k == 0), stop=False)
                mm(out=ps[:, s, :], lhsT=fcb_sb[:, VC * s:VC * (s + 1)],
                   rhs=ones64[:, :], start=False, stop=True)
            lo = spool.tile([VC, NCHK, NTOK], f32, tag="lo", bufs=2)
            if j % 2 == 0:
                nc.scalar.copy(lo[:, :, :], ps[:, :, :])
            else:
                nc.vector.tensor_copy(out=lo[:, :, :], in_=ps[:, :, :])
            nc.sync.dma_start(out=logits_d[j, :, :, :], in_=lo[:, :, :])
        fpsum.release()
        fcwpool.release()
        nc.leave_named_scope("fc", _sc_fc, False)
        mark("fc")
        spool.release()
        dmapool.release()
        cpool.release()
    except _PhaseExit:
        pass

    nc.finalize()
    return nc


def _prep_shared(inputs):
    """Host-side weight layout prep (shared across cores)."""
    import ml_dtypes
    bf = ml_dtypes.bfloat16
    f = np.float32
    perm = _gate_perm()
    w1 = inputs["cw1"].astype(f)
    w1b = w1.transpose(2, 3, 1, 0).reshape(27, 64)
    # block-diagonal stacked-images conv1 weight + bias row (K=55)
    w1s = np.zeros((55, 128), f)
    w1s[0:27, 0:64] = w1b
    w1s[27:54, 64:128] = w1b
    w1s[54, 0:64] = inputs["cb1"].astype(f)
    w1s[54, 64:128] = inputs["cb1"].astype(f)
    cb2t = inputs["cb2"].astype(f).reshape(128, 1).copy()
    w2t9 = inputs["cw2"].astype(f).transpose(2, 3, 1, 0).reshape(9, 64, 128)
    # tap pairs per image: im0 = [ky0; ky1] (dup half holds row+1), im1 = [ky1; ky0]
    w2p0 = np.concatenate([w2t9[0:3], w2t9[3:6]], axis=1)   # [3, 128, 128]
    w2p1 = np.concatenate([w2t9[3:6], w2t9[0:3]], axis=1)
    w2p = np.stack([w2p0, w2p1]).transpose(0, 2, 1, 3).copy()  # [2, 128, 3, 128]
    # singles (ky=2) duplicated in both partition halves
    w2s = np.concatenate([w2t9[6:9], w2t9[6:9]], axis=1).transpose(1, 0, 2).copy()
    w3t9 = inputs["cw3"].astype(f).transpose(2, 3, 1, 0).reshape(9, 128, 256)
    # [ky*kx=9, k2=2, 128, 512] -> [p=128, m=4, k2=2, tap=9, o=128]
    w4t9 = (inputs["cw4"].astype(f).transpose(2, 3, 1, 0).reshape(9, 2, 128, 4, 128)
            .transpose(2, 3, 1, 0, 4).copy())
    cb3t = inputs["cb3"].astype(f).reshape(2, 128).T.copy()
    cb4t = inputs["cb4"].astype(f).reshape(4, 128).T.copy()
    encwt = inputs["enc_w"].astype(f).T.reshape(4, 128, HID).copy()
    encbt = inputs["enc_b"].astype(f).reshape(5, 128).T.copy()
    attnwt = inputs["attn_w"].astype(f).T.reshape(10, 128, HID).copy()
    attnb = inputs["attn_b"].astype(f)[None, :]
    import concourse.mybir as _mb
    f8np = _mb.dt.np(_mb.dt.float8e4)
    wih = inputs["w_ih"].astype(f)[perm]
    whh = inputs["w_hh"].astype(f)[perm]
    # wih.T [2H, 4H] -> [m=20, p=128(k), kk=10, g=128]
    wiht = wih.T.reshape(10, 128, 20, 128).transpose(2, 1, 0, 3).copy()
    whht = whh.T.reshape(5, 128, 4 * HID).copy()
    bgate = (inputs["b_ih"].astype(f) + inputs["b_hh"].astype(f))[perm][None, :]
    fcwt = inputs["fc_w"].astype(f).T.reshape(5, 128, VOCAB).copy()
    fcb = inputs["fc_b"].astype(f)[None, :]
    bsel = np.zeros((BL, NTOK), f)
    for p in range(NTOK):
        bsel[p % BL, p] = 1.0
    return dict(w1s=w1s.astype(bf), cb2t=cb2t,
                w2p=w2p.astype(bf), w2s=w2s.astype(bf),
                w3t9=w3t9.astype(bf), w4t9=w4t9.astype(bf),
                cb3t=cb3t, cb4t=cb4t, encwt=encwt, encbt=encbt,
                attnwt=attnwt.astype(bf), attnb=attnb.astype(bf),
                wiht=wiht.astype(bf), whht=whht.astype(bf), bgate=bgate.astype(bf),
                fcwt=fcwt.astype(bf), fcb=fcb.astype(bf), bsel=bsel,
                emb=inputs["emb"].astype(f).astype(bf))


def _make_in_maps(inputs):
    shared = _prep_shared(inputs)
    images = np.asarray(inputs["images"], np.float32)
    captions = np.asarray(inputs["captions"])

    import ml_dtypes
    imgp = np.zeros((16, 3, 226, 226), np.float32)
    imgp[:, :, 1:225, 1:225] = images
    s = imgp.strides
    win = np.lib.stride_tricks.as_strided(
        imgp, shape=(16, 3, 3, 3, 224, 224),
        strides=(s[0], s[1], s[2], s[3], s[2], s[3]))
    # rows (ky, kx, c) to match w1 layout
    imcol = win.transpose(0, 2, 3, 1, 4, 5).reshape(16, 27, 224 * 224)
    imgp = imcol.astype(ml_dtypes.bfloat16)
    ones_row = np.ones((1, 224 * 224), ml_dtypes.bfloat16)
    in_maps = []
    for c in range(NCORES):
        caps = captions[BL * c:BL * (c + 1)].astype(np.int64).T.reshape(NTOK, 1)
        m = dict(shared)
        m["img"] = np.concatenate(
            [imgp[BL * c], imgp[BL * c + 1], ones_row], axis=0)
        m["caps"] = caps.astype(np.int32)
        in_maps.append(m)
    return in_maps


def kernel(**inputs):
    from concourse.bass_utils import run_bass_kernel_spmd

    if "nc" not in _NC_CACHE:
        _NC_CACHE["nc"] = build_bass()
    nc = _NC_CACHE["nc"]

    in_maps = _make_in_maps(inputs)
    res = run_bass_kernel_spmd(nc, in_maps, list(range(NCORES)))
    # logits come back as [j=10, v=125, s=8, tok]: vocab index = j*1000+s*125+v
    out = np.concatenate(
        [res.results[c]["logits"].transpose(0, 2, 1, 3)
             .reshape(VOCAB, T, BL).transpose(2, 1, 0)
         for c in range(NCORES)], axis=0)
    return out



# revision 80
# speedup vs baseline: 1.1042x; 1.0082x over previous
"""Trainium2 Bass kernel for CNN-encoder + attention-LSTM captioner + vocab FC.

Sharding: pure data-parallel over batch (16 images -> 8 cores x 2 images).
All weights replicated; no collectives. Host slices inputs / concatenates
outputs (logits come back vocab-major and are untransposed on the host).

Design notes (per core, BL=2 images, T=32 steps, all compute bf16/f32):
  - conv1 runs ONCE for both images: channels of im0 sit in partitions 0-63
    and im1 in 64-127 via a block-diagonal [55,128] weight (27 im2col rows per
    image + a ones row that folds in the bias). Pooling is
    relu(maxpool) == max(psum_even, relu(psum_odd)) rowmaxed into a staging
    tile; SBUF->SBUF DMAs then assemble per-image "tap pair" layouts
    [ch | ch shifted one row] so conv2 contracts K=128 for 6 of 9 taps.
  - conv2/3/4 are shift-accumulate 3x3 convs at the PE roofline; conv4's
    spatial mean rides the activation's accum_out.
  - the whole attention/gates precompute is batched over all 64 tokens and
    kept TRANSPOSED (gates on partitions): P.T = w_ih @ [e;ctx].T + b.
  - each LSTM step seeds PSUM with P_t via an identity matmul, accumulates
    W_hh @ h.T (weights stationary, h streaming N=2), and runs all gate
    nonlinearities/elementwise on [128, 5..20, 2] tiles so every engine lane
    is used; h.T lands directly in the FC-ready outsT buffer (no transposes).
  - FC computes logits.T (vocab rows on partitions, M=125) with fc_w
    prefetched during the LSTM; output DRAM layout is DMA-friendly
    [10, 125, 8, 64] and untransposed on the host.
  - big weights stream behind the image DMAs; w_ih prefetches into a scoped
    pool that is released before the fc_w stream needs the SBUF.
"""

import os
import numpy as np

os.environ.setdefault("MYCRO_LOCAL_CACHE", "1")

HID = 640
VOCAB = 10000
T = 32
BL = 2            # local batch per core
NTOK = T * BL     # 64
NCORES = 8

F32 = None  # set lazily (mybir.dt.float32)


class _PhaseExit(Exception):
    def __init__(self, tc):
        self.tc = tc

_NC_CACHE = {}
PHASE_MARKS = []   # (phase_name, inst_count_at_phase_end) recorded during build


def _gate_perm():
    # reference gate order [i, f, g, o] -> kernel order [i, f, o, g]
    return np.concatenate([
        np.arange(0, 1280),          # i, f
        np.arange(1920, 2560),       # o
        np.arange(1280, 1920),       # g
    ])


def build_bass(upto=None):
    import os
    upto = upto or os.environ.get("KERNEL_UPTO", "all")
    import concourse.bass as bass
    from concourse import bacc
    import concourse.tile_sem_assignment as tsa
    # Cap HWDGE sem lanes so pool-transition fan-ins stay under the
    # per-instruction sync-wait slot limits in walrus codegen.
    tsa.NUM_HWDGE_SEMS = 4
    import concourse.mybir as mybir
    import concourse.tile as tile
    from concourse.masks import make_identity

    f32 = mybir.dt.float32
    i32 = mybir.dt.int32
    AF = mybir.ActivationFunctionType
    ALU = mybir.AluOpType
    AX = mybir.AxisListType

    nc = bacc.Bacc(None)
    bf16 = mybir.dt.bfloat16

    PHASE_MARKS.clear()

    def mark(name):
        PHASE_MARKS.append((name, len(nc.inst_map)))

    def mm(out, lhsT, rhs, **kw):
        nc.tensor.matmul(out=out, lhsT=lhsT, rhs=rhs, **kw)

    # ---------------- DRAM parameters ----------------
    img_d = nc.declare_dram_parameter("img", [55, 224 * 224], bf16, isOutput=False)
    caps_d = nc.declare_dram_parameter("caps", [NTOK, 1], i32, isOutput=False)
    w1s_d = nc.declare_dram_parameter("w1s", [55, 128], bf16, isOutput=False)
    cb2_d = nc.declare_dram_parameter("cb2t", [128, 1], f32, isOutput=False)
    w2p_d = nc.declare_dram_parameter("w2p", [2, 128, 3, 128], bf16, isOutput=False)
    w2s_d = nc.declare_dram_parameter("w2s", [128, 3, 128], bf16, isOutput=False)
    w3t9_d = nc.declare_dram_parameter("w3t9", [9, 128, 256], bf16, isOutput=False)
    w4t9_d = nc.declare_dram_parameter("w4t9", [128, 4, 2, 9, 128], bf16, isOutput=False)
    cb3_d = nc.declare_dram_parameter("cb3t", [128, 2], f32, isOutput=False)
    cb4_d = nc.declare_dram_parameter("cb4t", [128, 4], f32, isOutput=False)
    encw_d = nc.declare_dram_parameter("encwt", [4, 128, HID], f32, isOutput=False)
    encb_d = nc.declare_dram_parameter("encbt", [128, 5], f32, isOutput=False)
    emb_d = nc.declare_dram_parameter("emb", [VOCAB, HID], bf16, isOutput=False)
    attnw_d = nc.declare_dram_parameter("attnwt", [10, 128, HID], bf16, isOutput=False)
    attnb_d = nc.declare_dram_parameter("attnb", [1, HID], bf16, isOutput=False)
    wih_d = nc.declare_dram_parameter("wiht", [20, 128, 10, 128], bf16, isOutput=False)
    whh_d = nc.declare_dram_parameter("whht", [5, 128, 4 * HID], bf16, isOutput=False)
    bgate_d = nc.declare_dram_parameter("bgate", [1, 4 * HID], bf16, isOutput=False)
    fcw_d = nc.declare_dram_parameter("fcwt", [5, 128, VOCAB], bf16, isOutput=False)
    fcb_d = nc.declare_dram_parameter("fcb", [1, VOCAB], bf16, isOutput=False)
    bsel_d = nc.declare_dram_parameter("bsel", [BL, NTOK], f32, isOutput=False)
    # logits stored transposed + group-blocked: [group, vocab_row, chunk, tok]
    NVC = 80          # vocab chunks
    VC = VOCAB // NVC  # 125 vocab rows per chunk
    logits_d = nc.declare_dram_parameter("logits", [10, VC, 8, NTOK], f32,
                                         isOutput=True)

    try:
      with tile.TileContext(nc) as tc:
        # ---------------- persistent constants ----------------
        cpool = tc.alloc_tile_pool(name="const", bufs=1)
        # pool for all DMA-written tiles: never released mid-kernel so that
        # SBUF zone reuse never makes compute ops wait on DMA queue sems
        dmapool = tc.alloc_tile_pool(name="dmat", bufs=1)
        ident = cpool.tile([128, 128], f32)
        make_identity(nc, ident[:, :])
        identb = cpool.tile([128, 128], bf16)
        make_identity(nc, identb[:, :])
        ones64 = cpool.tile([1, 64], bf16)
        nc.gpsimd.memset(ones64[:, :], 1.0)
        bsel_sb = dmapool.tile([BL, NTOK], f32)
        nc.sync.dma_start(out=bsel_sb[:, :], in_=bsel_d[:, :])
        feat_sb = cpool.tile([128, 4, BL], f32)   # feat.T, K-chunked [128,4] per img

        w1s_sb = dmapool.tile([55, 128], bf16)
        nc.sync.dma_start(out=w1s_sb[:, :], in_=w1s_d[:, :])
        cb2_sb = dmapool.tile([128, 1], f32)
        nc.sync.dma_start(out=cb2_sb[:, :], in_=cb2_d[:, :])
        w2p_sb = dmapool.tile([128, 2, 3, 128], bf16)
        nc.sync.dma_start(out=w2p_sb[:, :, :, :],
                          in_=w2p_d[:, :, :, :].rearrange("i p t o -> p i t o"))
        w2s_sb = dmapool.tile([128, 3, 128], bf16)
        nc.sync.dma_start(out=w2s_sb[:, :, :], in_=w2s_d[:, :, :])
        w3_sb = dmapool.tile([128, 9, 256], bf16)
        nc.sync.dma_start(out=w3_sb[:, :, :], in_=w3t9_d[:, :, :].rearrange("t p o -> p t o"))
        cb3_sb = dmapool.tile([128, 2], f32)
        nc.sync.dma_start(out=cb3_sb[:, :], in_=cb3_d[:, :])
        cb4_sb = dmapool.tile([128, 4], f32)
        nc.sync.dma_start(out=cb4_sb[:, :], in_=cb4_d[:, :])

        # ---------------- conv tower ----------------
        # per-image padded intermediates; only borders need zeroing (interiors
        # are fully rewritten). x2 tiles live in their own pool so their SBUF
        # frees right after conv2(im1), making room to prefetch w_ih early.
        spool = tc.alloc_tile_pool(name="seq", bufs=1)
        padpool = tc.alloc_tile_pool(name="pads", bufs=1)
        x2pool = tc.alloc_tile_pool(name="x2p", bufs=1)
        # staging: [im0ch | im1ch] padded pool1 output + an extra always-zero
        # row 114 so the row+1-shifted dup copies need no edge memset
        x2s = x2pool.tile([128, 115, 114], bf16)
        nc.vector.memset(x2s[:, 0, :], 0.0)
        nc.vector.memset(x2s[:, 113:115, :], 0.0)
        nc.vector.memset(x2s[:, :, 0], 0.0)
        nc.vector.memset(x2s[:, :, 113], 0.0)
        x2im = []
        for im in range(BL):
            x2t = x2pool.tile([128, 114, 114], bf16)
            x2im.append(x2t)
        x3im = [None, None]
        x4im = [None, None]

        # ---- conv1 (3->64), both images stacked on partitions (M=64ch x 2im),
        # im2col K=54 + ones-row bias; pool chain: DVE xmax (psum evict),
        # Pool rowmax + relu ----
        _sc1 = nc.enter_named_scope("conv1", False)[0]
        c1pool = tc.alloc_tile_pool(name="c1", bufs=4)
        c1psum = tc.alloc_tile_pool(name="c1p", bufs=4, space="PSUM")
        R = 16
        for ch in range(224 // R):
            Y = R * ch
            rh = dmapool.tile([55, R * 224], bf16, tag="rh", bufs=2)
            nc.sync.dma_start(out=rh[:, :],
                              in_=img_d[:, Y * 224:(Y + R) * 224])
            rhv = rh.rearrange("p (r x) -> p r x", x=224)
            for q in range(4):
                ps = c1psum.tile([128, 2, 448], f32, padded_shape=[128, 2, 512],
                                 tag="ps")
                for s in range(2):
                    r0 = q * 4 + s * 2
                    mm(out=ps[:, s, :], lhsT=w1s_sb[:, :],
                       rhs=rhv[:, r0:r0 + 2, :], start=True, stop=True)
                # relu(maxpool) == max(even, relu(odd), rows): ACT relu-evicts
                # only the odd columns; DVE maxes them against the raw PSUM
                # evens (xm >= 0 always), then row-pools into x2s.
                a1 = c1pool.tile([128, 2, 224], bf16, tag="a1")
                nc.scalar.activation(a1[:, :, :], ps[:, :, 1:448:2], AF.Relu)
                xm = c1pool.tile([128, 2, 224], bf16, tag="xm")
                nc.vector.tensor_tensor(out=xm[:, :, :],
                                        in0=ps[:, :, 0:448:2],
                                        in1=a1[:, :, :], op=ALU.max)
                oy = (R * ch + 4 * q) // 2
                nc.vector.tensor_tensor(out=x2s[:, oy + 1:oy + 3, 1:113],
                                        in0=xm[:, :, 0:112],
                                        in1=xm[:, :, 112:224], op=ALU.max)
        w4_sb = dmapool.tile([128, 4, 2, 9, 128], bf16)
        nc.sync.dma_start(out=w4_sb[:, :, :, :, :], in_=w4t9_d[:, :, :, :, :])
        encw_sb = dmapool.tile([128, 4, HID], f32)
        nc.sync.dma_start(out=encw_sb[:, :, :], in_=encw_d[:, :, :].rearrange("k p o -> p k o"))
        encb_sb = dmapool.tile([128, 5], f32)
        nc.sync.dma_start(out=encb_sb[:, :], in_=encb_d[:, :])
        attnw_sb = dmapool.tile([128, 10, HID], bf16)
        nc.sync.dma_start(out=attnw_sb[:, :, :],
                          in_=attnw_d[:, :, :].rearrange("k p o -> p k o"))
        attnb_sb = dmapool.tile([1, HID], bf16)
        nc.sync.dma_start(out=attnb_sb[:, :], in_=attnb_d[:, :])
        whh_sb = dmapool.tile([128, 5, 4 * HID], bf16)
        nc.sync.dma_start(out=whh_sb[:, :, :],
                          in_=whh_d[:, :, :].rearrange("k p o -> p k o"))
        bgate_sb = dmapool.tile([1, 4 * HID], bf16)
        nc.sync.dma_start(out=bgate_sb[:, :], in_=bgate_d[:, :])
        idx_sb = dmapool.tile([NTOK, 1], i32)
        nc.sync.dma_start(out=idx_sb[:, :], in_=caps_d[:, :])
        e_sb = dmapool.tile([NTOK, HID], bf16)
        nc.gpsimd.indirect_dma_start(
            out=e_sb[:, :], out_offset=None,
            in_=emb_d[:, :],
            in_offset=bass.IndirectOffsetOnAxis(ap=idx_sb[:, :1], axis=0),
        )
        c1psum.release()
        c1pool.release()
        # assemble per-image tap-pair layouts from the staging tile:
        # x2im0 = [im0 | im0 shifted+1], x2im1 = [im1 shifted+1 | im1]
        for r0, r1 in ((0, 57), (57, 114)):
            nc.sync.dma_start(out=x2im[0][0:64, r0:r1, :],
                              in_=x2s[0:64, r0:r1, :])
            nc.sync.dma_start(out=x2im[0][64:128, r0:r1, :],
                              in_=x2s[0:64, r0 + 1:r1 + 1, :])
            nc.sync.dma_start(out=x2im[1][64:128, r0:r1, :],
                              in_=x2s[64:128, r0:r1, :])
            nc.sync.dma_start(out=x2im[1][0:64, r0:r1, :],
                              in_=x2s[64:128, r0 + 1:r1 + 1, :])
        nc.leave_named_scope("conv1", _sc1, False)
        mark("conv1")
        NG = 4 * HID // 128   # 20 gate chunks: i=0-4, f=5-9, o=10-14, g=15-19
        wih_sb = None
        for im in range(BL):
            _sc = nc.enter_named_scope(f"conv_im{im}", False)[0]

            # ---- conv2 (64->128): 3 K=128 tap-pairs + 3 K=64 singles ----
            x3t = padpool.tile([128, 58, 58], bf16)
            nc.vector.memset(x3t[:, :, :], 0.0)
            x3im[im] = x3t
            x4t = padpool.tile([128, 2, 30, 30], bf16)
            nc.vector.memset(x4t[:, :, :, :], 0.0)
            x4im[im] = x4t
            c2psum = tc.alloc_tile_pool(name=f"c2p_{im}", bufs=3, space="PSUM")
            c2pool = tc.alloc_tile_pool(name=f"c2_{im}", bufs=2)
            sb = 0 if im == 0 else 64   # singles partition base for this image
            for tl in range(14):  # 8 output rows per tile
                ps = c2psum.tile([128, 2, 448], f32, padded_shape=[128, 2, 512], tag="ps")
                for s in range(2):
                    y0 = tl * 8 + s * 4
                    for kx in range(3):
                        mm(
                            out=ps[:, s, :], lhsT=w2p_sb[:, im, kx, :],
                            rhs=x2im[im][:, y0:y0 + 4, kx:kx + 112],
                            start=(kx == 0), stop=False,
                        )
                    for kx in range(3):
                        mm(
                            out=ps[:, s, :], lhsT=w2s_sb[sb:sb + 64, kx, :],
                            rhs=x2im[im][sb:sb + 64, y0 + 2:y0 + 6, kx:kx + 112],
                            start=False, stop=(kx == 2),
                        )
                a2 = c2pool.tile([128, 8, 112], bf16, tag="a2")
                nc.scalar.activation(
                    a2.rearrange("p (a y) x -> p a y x", a=2),
                    ps.rearrange("p a (y x) -> p a y x", x=112),
                    AF.Relu, bias=cb2_sb[:, 0:1])
                t2 = c2pool.tile([128, 8, 56], bf16, tag="t2")
                nc.vector.tensor_tensor(
                    out=t2[:, :, :], in0=a2[:, :, 0:112:2], in1=a2[:, :, 1:112:2],
                    op=ALU.max,
                )
                nc.vector.tensor_tensor(
                    out=x3im[im][:, tl * 4 + 1:tl * 4 + 5, 1:57],
                    in0=t2[:, 0:8:2, :], in1=t2[:, 1:8:2, :],
                    op=ALU.max,
                )
            c2psum.release()
            c2pool.release()
            if im == BL - 1:
                # x2 SBUF is dead: free it and prefetch the full w_ih under
                # the remaining conv3/conv4 compute
                x2pool.release()
                wihpool = tc.alloc_tile_pool(name="wihp", bufs=1)
                wih_sb = wihpool.tile([128, NG, 10, 128], bf16)
                for g in range(4):
                    nc.sync.dma_start(
                        out=wih_sb[:, 5 * g:5 * (g + 1), :, :],
                        in_=wih_d[5 * g:5 * (g + 1), :, :, :].rearrange(
                            "m p k o -> p m k o"))

            # ---- conv3 (128->256) K=128, bias via ACT evict, pool -> x4_pad ----
            c3psum = tc.alloc_tile_pool(name=f"c3p_{im}", bufs=3, space="PSUM")
            c3pool = tc.alloc_tile_pool(name=f"c3_{im}", bufs=2)
            for m in range(2):
                for tl in range(7):  # 8 output rows per tile
                    ps = c3psum.tile([128, 448], f32, padded_shape=[128, 512], tag="ps")
                    y0 = tl * 8
                    for ky in range(3):
                        for kx in range(3):
                            tap = ky * 3 + kx
                            rhs = x3im[im][:, y0 + ky:y0 + ky + 8, kx:kx + 56]
                            mm(
                                out=ps[:, :],
                                lhsT=w3_sb[:, tap, 128 * m:128 * (m + 1)],
                                rhs=rhs,
                                start=(tap == 0), stop=(tap == 8),
                            )
                    a3 = c3pool.tile([128, 8, 56], bf16, tag="a3")
                    nc.scalar.activation(
                        a3[:, :, :],
                        ps.rearrange("p (y x) -> p y x", x=56),
                        AF.Relu, bias=cb3_sb[:, m:m + 1])
                    t3 = c3pool.tile([128, 8, 28], bf16, tag="t3")
                    nc.vector.tensor_tensor(
                        out=t3[:, :, :], in0=a3[:, :, 0:56:2], in1=a3[:, :, 1:56:2],
                        op=ALU.max,
                    )
                    nc.vector.tensor_tensor(
                        out=x4im[im][:, m, tl * 4 + 1:tl * 4 + 5, 1:29],
                        in0=t3[:, 0:8:2, :], in1=t3[:, 1:8:2, :],
                        op=ALU.max,
                    )
            c3psum.release()
            c3pool.release()

            # ---- conv4 (256->512) K=256 (2 chunks), no pool; mean via accum_out ----
            ipool = tc.alloc_tile_pool(name=f"img{im}", bufs=1)
            c4psum = tc.alloc_tile_pool(name=f"c4p_{im}", bufs=3, space="PSUM")
            c4pool = tc.alloc_tile_pool(name=f"c4_{im}", bufs=2)
            msum = ipool.tile([128, 4, 2], f32)
            for m in range(4):
                w4m = w4_sb[:, m]
                ps = c4psum.tile([128, 2, 392], f32, padded_shape=[128, 2, 512], tag="ps")
                for s in range(2):
                    y0 = s * 14
                    first = True
                    for ky in range(3):
                        for kx in range(3):
                            tap = ky * 3 + kx
                            for k2 in range(2):
                                rhs = x4im[im][:, k2, y0 + ky:y0 + ky + 14, kx:kx + 28]
                                mm(
                                    out=ps[:, s, :],
                                    lhsT=w4m[:, k2, tap, :],
                                    rhs=rhs,
                                    start=first, stop=(tap == 8 and k2 == 1),
                                )
                                first = False
                a4 = c4pool.tile([128, 2, 392], bf16, tag="a4")
                for s in range(2):
                    nc.scalar.activation(a4[:, s, :], ps[:, s, :], AF.Relu,
                                         bias=cb4_sb[:, m:m + 1],
                                         accum_out=msum[:, m, s:s + 1])
            c4psum.release()
            c4pool.release()
            # feat.T[:, m] = (msum[:,m,0] + msum[:,m,1]) / 784
            tmpf = ipool.tile([128, 4], f32)
            nc.vector.tensor_tensor(out=tmpf[:, :], in0=msum[:, :, 0], in1=msum[:, :, 1],
                                    op=ALU.add)
            nc.vector.tensor_scalar_mul(feat_sb[:, :, im], tmpf[:, :], 1.0 / 784.0)
            ipool.release()
            nc.leave_named_scope(f"conv_im{im}", _sc, False)
            mark(f"conv_im{im}")

        if upto == "conv":
            raise _PhaseExit(tc)

        # ---------------- encoder linear: memory.T = enc_w @ feat.T + enc_b ----------------
        _sc_ea = nc.enter_named_scope("enc_attn", False)[0]
        scpool = tc.alloc_tile_pool(name="scratch", bufs=1)
        p1psum = tc.alloc_tile_pool(name="p1ps", bufs=1, space="PSUM")
        memT_ps = p1psum.tile([128, 5, BL], f32)
        for m in range(5):
            for k in range(4):
                nc.tensor.matmul(
                    out=memT_ps[:, m, :],
                    lhsT=encw_sb[:, k, 128 * m:128 * (m + 1)],
                    rhs=feat_sb[:, k, :],
                    start=(k == 0), stop=(k == 3),
                )
        memT_sb = spool.tile([128, 5, BL], f32)
        for m in range(5):
            nc.vector.tensor_scalar_add(memT_sb[:, m, :], memT_ps[:, m, :],
                                        encb_sb[:, m:m + 1])
        # memory non-transposed [2, 640]
        mem_ps = p1psum.tile([BL, HID], f32)
        for m in range(5):
            nc.tensor.transpose(out=mem_ps[:, 128 * m:128 * (m + 1)],
                                in_=memT_sb[:, m, :], identity=ident[:, :])
        mem_sb = scpool.tile([BL, HID], f32)
        nc.scalar.copy(mem_sb[:, :], mem_ps[:, :])

        # memory broadcast to all tokens [64, 640] via bsel matmul
        mexp_ps = p1psum.tile([NTOK, HID], f32)
        for n in range(2):
            sl = slice(512 * n, min(HID, 512 * (n + 1)))
            nc.tensor.matmul(out=mexp_ps[:, sl], lhsT=bsel_sb[:, :], rhs=mem_sb[:, sl],
                             start=True, stop=True)
        mexp_sb = scpool.tile([NTOK, HID], f32)
        nc.scalar.copy(mexp_sb[:, :], mexp_ps[:, :])
        p1psum.release()
        p1bpsum = tc.alloc_tile_pool(name="p1bps", bufs=1, space="PSUM")

        # fusedT [128, 10, 64]: chunks 0-4 = e.T ; 5-9 = memory.T broadcast
        fusedT_pse = p1bpsum.tile([128, 5, NTOK], bf16)
        for k in range(5):
            nc.tensor.transpose(out=fusedT_pse[:, k, :],
                                in_=e_sb[:, 128 * k:128 * (k + 1)],
                                identity=identb[0:64, 0:64])
        fusedT_psm = p1bpsum.tile([128, 5, NTOK], f32)
        for m in range(5):
            nc.tensor.matmul(out=fusedT_psm[:, m, :],
                             lhsT=mem_sb[:, 128 * m:128 * (m + 1)],
                             rhs=bsel_sb[:, :], start=True, stop=True)
        fusedT_sb = spool.tile([128, 10, NTOK], bf16)
        nc.scalar.copy(fusedT_sb[:, 0:5, :], fusedT_pse[:, :, :])
        nc.scalar.copy(fusedT_sb[:, 5:10, :], fusedT_psm[:, :, :])

        # ---------------- attention (batched over all tokens) ----------------
        attnw_sb = dmapool.tile([128, 10, HID], bf16)
        nc.sync.dma_start(out=attnw_sb[:, :, :],
                          in_=attnw_d[:, :, :].rearrange("k p o -> p k o"))
        attnb_sb = dmapool.tile([1, HID], bf16)
        nc.sync.dma_start(out=attnb_sb[:, :], in_=attnb_d[:, :])

        attn_ps = p1bpsum.tile([NTOK, HID], f32)
        for n in range(2):
            sl = slice(512 * n, min(HID, 512 * (n + 1)))
            for k in range(10):
                mm(out=attn_ps[:, sl], lhsT=fusedT_sb[:, k, :],
                   rhs=attnw_sb[:, k, sl], start=(k == 0), stop=False)
            mm(out=attn_ps[:, sl], lhsT=ones64[:, :],
               rhs=attnb_sb[:, sl], start=False, stop=True)
        # softmax over free dim, then context = softmax * memory
        nmx_sb = scpool.tile([NTOK, 1], f32)
        nc.vector.reduce_max(out=nmx_sb[:, :], in_=attn_ps[:, :], axis=AX.X,
                             negate=True)
        ex_sb = scpool.tile([NTOK, HID], f32)
        ssum_sb = scpool.tile([NTOK, 1], f32)
        nc.scalar.activation(ex_sb[:, :], attn_ps[:, :], AF.Exp,
                             bias=nmx_sb[:, 0:1], accum_out=ssum_sb[:, 0:1])
        rcp_sb = scpool.tile([NTOK, 1], f32)
        nc.vector.reciprocal(rcp_sb[:, :], ssum_sb[:, :])
        ctx_sb = scpool.tile([NTOK, HID], bf16)
        nc.vector.tensor_scalar_mul(ctx_sb[:, :], ex_sb[:, :], rcp_sb[:, 0:1])
        nc.vector.tensor_tensor(out=ctx_sb[:, :], in0=ctx_sb[:, :], in1=mexp_sb[:, :],
                                op=ALU.mult)
        ctxT_ps = p1bpsum.tile([128, 5, NTOK], bf16)
        for k in range(5):
            nc.tensor.transpose(out=ctxT_ps[:, k, :],
                                in_=ctx_sb[:, 128 * k:128 * (k + 1)],
                                identity=identb[0:64, 0:64])
        ctxT_sb = spool.tile([128, 5, NTOK], bf16)
        nc.scalar.copy(ctxT_sb[:, :, :], ctxT_ps[:, :, :])
        p1bpsum.release()
        scpool.release()

        p2psum = tc.alloc_tile_pool(name="p2ps", bufs=1, space="PSUM")
        PT_ps = p2psum.tile([128, NG, NTOK], f32)
        for m in range(NG):
            for k in range(10):
                rhsT = fusedT_sb[:, k, :] if k < 5 else ctxT_sb[:, k - 5, :]
                mm(out=PT_ps[:, m, :], lhsT=wih_sb[:, m, k, :],
                   rhs=rhsT, start=(k == 0), stop=False)
            mm(out=PT_ps[:, m, :], lhsT=bgate_sb[:, 128 * m:128 * (m + 1)],
               rhs=ones64[:, :], start=False, stop=True)
        PT_sb = spool.tile([128, NG, NTOK], bf16)
        nc.scalar.copy(PT_sb[:, :, :], PT_ps[:, :, :])
        p2psum.release()
        wihpool.release()
        padpool.release()
        nc.leave_named_scope("enc_attn", _sc_ea, False)
        mark("enc_attn")

        if upto == "pre":
            raise _PhaseExit(tc)
        _sc_ls = nc.enter_named_scope("lstm", False)[0]
        # ---------------- LSTM recurrence (transposed: gates.T on partitions) ----------------
        outsT_sb = spool.tile([128, 5, NTOK], bf16)   # h.T for every step
        c_sb = spool.tile([128, 5, BL], f32)

        # FC weight stream: allocate + DMA before the LSTM so transfers overlap it
        fcwpool = tc.alloc_tile_pool(name="fcw", bufs=1)
        lpsum = tc.alloc_tile_pool(name="lstm_ps", bufs=2, space="PSUM")
        lsp = tc.alloc_tile_pool(name="lstm_sb", bufs=2)
        CH = 1000
        fws = []
        for j in range(VOCAB // CH):
            fw = fcwpool.tile([128, 5, CH], bf16, tag="fw", bufs=10)
            nc.sync.dma_start(out=fw[:, :, :],
                              in_=fcw_d[:, :, CH * j:CH * (j + 1)].rearrange(
                                  "k p o -> p k o"))
            fcb_sb = fcwpool.tile([1, CH], bf16, tag="fcb", bufs=2)
            nc.sync.dma_start(out=fcb_sb[:, :], in_=fcb_d[:, CH * j:CH * (j + 1)])
            fws.append((fw, fcb_sb))
        for t in range(T):
            tt = slice(BL * t, BL * (t + 1))
            if t == 0:
                gsum = PT_sb
                gt = tt
            else:
                gt = slice(0, BL)
                gatesT_ps = lpsum.tile([128, NG, BL], f32, tag="gates")
                for m in range(NG):
                    # seed the accumulator with P_t via identity matmul, then
                    # accumulate W_hh @ h
                    mm(out=gatesT_ps[:, m, :], lhsT=identb[:, :],
                       rhs=PT_sb[:, m, tt], start=True, stop=False)
                    for k in range(5):
                        mm(out=gatesT_ps[:, m, :],
                           lhsT=whh_sb[:, k, 128 * m:128 * (m + 1)],
                           rhs=outsT_sb[:, k, BL * (t - 1):BL * t],
                           start=False, stop=(k == 4))
                gsum = gatesT_ps
            # one sigmoid covers every gate: host doubled the g-gate rows, so
            # tanh(g) = 2*sigmoid(2g) - 1 comes from a cheap DVE affine
            sig = lsp.tile([128, NG, BL], f32, tag="sig")
            nc.scalar.activation(sig[:, :, :], gsum[:, 0:NG, gt], AF.Sigmoid)
            tg = lsp.tile([128, 5, BL], f32, tag="tg")
            nc.vector.tensor_scalar(tg[:, :, :], sig[:, 15:20, :], 2.0, -1.0,
                                    ALU.mult, ALU.add)
            # f*c first: it only needs sig, so DVE overlaps ACT's tanh
            if t > 0:
                nc.vector.tensor_tensor(out=c_sb[:, :, :], in0=sig[:, 5:10, :],
                                        in1=c_sb[:, :, :], op=ALU.mult)
            ig = lsp.tile([128, 5, BL], f32, tag="ig")
            nc.vector.tensor_tensor(out=ig[:, :, :], in0=sig[:, 0:5, :],
                                    in1=tg[:, :, :], op=ALU.mult)
            if t > 0:
                nc.vector.tensor_tensor(out=c_sb[:, :, :], in0=c_sb[:, :, :],
                                        in1=ig[:, :, :], op=ALU.add)
            else:
                nc.vector.tensor_copy(out=c_sb[:, :, :], in_=ig[:, :, :])
            thc = lsp.tile([128, 5, BL], f32, tag="thc")
            nc.scalar.activation(thc[:, :, :], c_sb[:, :, :], AF.Tanh)
            nc.vector.tensor_tensor(out=outsT_sb[:, :, tt], in0=sig[:, 10:15, :],
                                    in1=thc[:, :, :], op=ALU.mult)
        lsp.release()
        lpsum.release()
        nc.leave_named_scope("lstm", _sc_ls, False)
        mark("lstm")

        if upto == "lstm":
            raise _PhaseExit(tc)
        _sc_fc = nc.enter_named_scope("fc", False)[0]
        # -------- FC to vocab (transposed): logits.T = fc_w @ outs.T + fc_b --------
        # vocab rows on partitions (M=125), tokens streaming (N=64); PSUM -> DRAM.
        fpsum = tc.alloc_tile_pool(name="fc_ps", bufs=2, space="PSUM")
        NCHK = CH // VC   # 8 vocab chunks per CH group = one full PSUM bank
        for j in range(VOCAB // CH):
            fw, fcb_sb = fws[j]
            ps = fpsum.tile([VC, NCHK, NTOK], f32, tag="ps")
            for s in range(NCHK):
                for k in range(5):
                    mm(out=ps[:, s, :], lhsT=fw[:, k, VC * s:VC * (s + 1)],
                       rhs=outsT_sb[:, k, :],
                       start=(k == 0), stop=False)
                mm(out=ps[:, s, :], lhsT=fcb_sb[:, VC * s:VC * (s + 1)],
                   rhs=ones64[:, :], start=False, stop=True)
            lo = spool.tile([VC, NCHK, NTOK], f32, tag="lo", bufs=2)
            if j % 2 == 0:
                nc.scalar.copy(lo[:, :, :], ps[:, :, :])
            else:
                nc.vector.tensor_copy(out=lo[:, :, :], in_=ps[:, :, :])
            nc.sync.dma_start(out=logits_d[j, :, :, :], in_=lo[:, :, :])
        fpsum.release()
        fcwpool.release()
        nc.leave_named_scope("fc", _sc_fc, False)
        mark("fc")
        spool.release()
        dmapool.release()
        cpool.release()
    except _PhaseExit:
        pass

    nc.finalize()
    return nc


def _prep_shared(inputs):
    """Host-side weight layout prep (shared across cores)."""
    import ml_dtypes
    bf = ml_dtypes.bfloat16
    f = np.float32
    perm = _gate_perm()
    w1 = inputs["cw1"].astype(f)
    w1b = w1.transpose(2, 3, 1, 0).reshape(27, 64)
    # block-diagonal stacked-images conv1 weight + bias row (K=55)
    w1s = np.zeros((55, 128), f)
    w1s[0:27, 0:64] = w1b
    w1s[27:54, 64:128] = w1b
    w1s[54, 0:64] = inputs["cb1"].astype(f)
    w1s[54, 64:128] = inputs["cb1"].astype(f)
    cb2t = inputs["cb2"].astype(f).reshape(128, 1).copy()
    w2t9 = inputs["cw2"].astype(f).transpose(2, 3, 1, 0).reshape(9, 64, 128)
    # tap pairs per image: im0 = [ky0; ky1] (dup half holds row+1), im1 = [ky1; ky0]
    w2p0 = np.concatenate([w2t9[0:3], w2t9[3:6]], axis=1)   # [3, 128, 128]
    w2p1 = np.concatenate([w2t9[3:6], w2t9[0:3]], axis=1)
    w2p = np.stack([w2p0, w2p1]).transpose(0, 2, 1, 3).copy()  # [2, 128, 3, 128]
    # singles (ky=2) duplicated in both partition halves
    w2s = np.concatenate([w2t9[6:9], w2t9[6:9]], axis=1).transpose(1, 0, 2).copy()
    w3t9 = inputs["cw3"].astype(f).transpose(2, 3, 1, 0).reshape(9, 128, 256)
    # [ky*kx=9, k2=2, 128, 512] -> [p=128, m=4, k2=2, tap=9, o=128]
    w4t9 = (inputs["cw4"].astype(f).transpose(2, 3, 1, 0).reshape(9, 2, 128, 4, 128)
            .transpose(2, 3, 1, 0, 4).copy())
    cb3t = inputs["cb3"].astype(f).reshape(2, 128).T.copy()
    cb4t = inputs["cb4"].astype(f).reshape(4, 128).T.copy()
    encwt = inputs["enc_w"].astype(f).T.reshape(4, 128, HID).copy()
    encbt = inputs["enc_b"].astype(f).reshape(5, 128).T.copy()
    attnwt = inputs["attn_w"].astype(f).T.reshape(10, 128, HID).copy()
    attnb = inputs["attn_b"].astype(f)[None, :]
    wih = inputs["w_ih"].astype(f)[perm]
    whh = inputs["w_hh"].astype(f)[perm]
    # g-gate rows x2 so tanh(g) = 2*sigmoid(2g) - 1 (exact bf16 scale)
    wih[1920:2560] *= 2.0
    whh[1920:2560] *= 2.0
    # wih.T [2H, 4H] -> [m=20, p=128(k), kk=10, g=128]
    wiht = wih.T.reshape(10, 128, 20, 128).transpose(2, 1, 0, 3).copy()
    whht = whh.T.reshape(5, 128, 4 * HID).copy()
    bgate = (inputs["b_ih"].astype(f) + inputs["b_hh"].astype(f))[perm][None, :].copy()
    bgate[:, 1920:2560] *= 2.0
    fcwt = inputs["fc_w"].astype(f).T.reshape(5, 128, VOCAB).copy()
    fcb = inputs["fc_b"].astype(f)[None, :]
    bsel = np.zeros((BL, NTOK), f)
    for p in range(NTOK):
        bsel[p % BL, p] = 1.0
    return dict(w1s=w1s.astype(bf), cb2t=cb2t,
                w2p=w2p.astype(bf), w2s=w2s.astype(bf),
                w3t9=w3t9.astype(bf), w4t9=w4t9.astype(bf),
                cb3t=cb3t, cb4t=cb4t, encwt=encwt, encbt=encbt,
                attnwt=attnwt.astype(bf), attnb=attnb.astype(bf),
                wiht=wiht.astype(bf), whht=whht.astype(bf), bgate=bgate.astype(bf),
                fcwt=fcwt.astype(bf), fcb=fcb.astype(bf), bsel=bsel,
                emb=inputs["emb"].astype(f).astype(bf))


def _make_in_maps(inputs):
    shared = _prep_shared(inputs)
    images = np.asarray(inputs["images"], np.float32)
    captions = np.asarray(inputs["captions"])

    import ml_dtypes
    imgp = np.zeros((16, 3, 226, 226), np.float32)
    imgp[:, :, 1:225, 1:225] = images
    s = imgp.strides
    win = np.lib.stride_tricks.as_strided(
        imgp, shape=(16, 3, 3, 3, 224, 224),
        strides=(s[0], s[1], s[2], s[3], s[2], s[3]))
    # rows (ky, kx, c) to match w1 layout
    imcol = win.transpose(0, 2, 3, 1, 4, 5).reshape(16, 27, 224 * 224)
    imgp = imcol.astype(ml_dtypes.bfloat16)
    ones_row = np.ones((1, 224 * 224), ml_dtypes.bfloat16)
    in_maps = []
    for c in range(NCORES):
        caps = captions[BL * c:BL * (c + 1)].astype(np.int64).T.reshape(NTOK, 1)
        m = dict(shared)
        m["img"] = np.concatenate(
            [imgp[BL * c], imgp[BL * c + 1], ones_row], axis=0)
        m["caps"] = caps.astype(np.int32)
        in_maps.append(m)
    return in_maps


def kernel(**inputs):
    from concourse.bass_utils import run_bass_kernel_spmd

    if "nc" not in _NC_CACHE:
        _NC_CACHE["nc"] = build_bass()
    nc = _NC_CACHE["nc"]

    in_maps = _make_in_maps(inputs)
    res = run_bass_kernel_spmd(nc, in_maps, list(range(NCORES)))
    # logits come back as [j=10, v=125, s=8, tok]: vocab index = j*1000+s*125+v
    out = np.concatenate(
        [res.results[c]["logits"].transpose(0, 2, 1, 3)
             .reshape(VOCAB, T, BL).transpose(2, 1, 0)
         for c in range(NCORES)], axis=0)
    return out



# revision 81
# speedup vs baseline: 1.1117x; 1.0068x over previous
"""Trainium2 Bass kernel for CNN-encoder + attention-LSTM captioner + vocab FC.

Sharding: pure data-parallel over batch (16 images -> 8 cores x 2 images).
All weights replicated; no collectives. Host slices inputs / concatenates
outputs (logits come back vocab-major and are untransposed on the host).

Design notes (per core, BL=2 images, T=32 steps, all compute bf16/f32):
  - conv1 runs ONCE for both images: channels of im0 sit in partitions 0-63
    and im1 in 64-127 via a block-diagonal [55,128] weight (27 im2col rows per
    image + a ones row that folds in the bias). Pooling is
    relu(maxpool) == max(psum_even, relu(psum_odd)) rowmaxed into a staging
    tile; SBUF->SBUF DMAs then assemble per-image "tap pair" layouts
    [ch | ch shifted one row] so conv2 contracts K=128 for 6 of 9 taps.
  - conv2/3/4 are shift-accumulate 3x3 convs at the PE roofline; conv4's
    spatial mean rides the activation's accum_out.
  - the whole attention/gates precompute is batched over all 64 tokens and
    kept TRANSPOSED (gates on partitions): P.T = w_ih @ [e;ctx].T + b.
  - each LSTM step seeds PSUM with P_t via an identity matmul, accumulates
    W_hh @ h.T (weights stationary, h streaming N=2), and runs all gate
    nonlinearities/elementwise on [128, 5..20, 2] tiles so every engine lane
    is used; h.T lands directly in the FC-ready outsT buffer (no transposes).
  - FC computes logits.T (vocab rows on partitions, M=125) with fc_w
    prefetched during the LSTM; output DRAM layout is DMA-friendly
    [10, 125, 8, 64] and untransposed on the host.
  - big weights stream behind the image DMAs; w_ih prefetches into a scoped
    pool that is released before the fc_w stream needs the SBUF.
"""

import os
import numpy as np

os.environ.setdefault("MYCRO_LOCAL_CACHE", "1")

HID = 640
VOCAB = 10000
T = 32
BL = 2            # local batch per core
NTOK = T * BL     # 64
NCORES = 8

F32 = None  # set lazily (mybir.dt.float32)


class _PhaseExit(Exception):
    def __init__(self, tc):
        self.tc = tc

_NC_CACHE = {}
PHASE_MARKS = []   # (phase_name, inst_count_at_phase_end) recorded during build


def _gate_perm():
    # reference gate order [i, f, g, o] -> kernel order [i, f, o, g]
    return np.concatenate([
        np.arange(0, 1280),          # i, f
        np.arange(1920, 2560),       # o
        np.arange(1280, 1920),       # g
    ])


def build_bass(upto=None):
    import os
    upto = upto or os.environ.get("KERNEL_UPTO", "all")
    import concourse.bass as bass
    from concourse import bacc
    import concourse.tile_sem_assignment as tsa
    # Cap HWDGE sem lanes so pool-transition fan-ins stay under the
    # per-instruction sync-wait slot limits in walrus codegen.
    tsa.NUM_HWDGE_SEMS = 4
    import concourse.mybir as mybir
    import concourse.tile as tile
    from concourse.masks import make_identity

    f32 = mybir.dt.float32
    i32 = mybir.dt.int32
    AF = mybir.ActivationFunctionType
    ALU = mybir.AluOpType
    AX = mybir.AxisListType

    nc = bacc.Bacc(None)
    bf16 = mybir.dt.bfloat16

    PHASE_MARKS.clear()

    def mark(name):
        PHASE_MARKS.append((name, len(nc.inst_map)))

    def mm(out, lhsT, rhs, **kw):
        nc.tensor.matmul(out=out, lhsT=lhsT, rhs=rhs, **kw)

    # ---------------- DRAM parameters ----------------
    img_d = nc.declare_dram_parameter("img", [55, 224 * 224], bf16, isOutput=False)
    caps_d = nc.declare_dram_parameter("caps", [NTOK, 1], i32, isOutput=False)
    w1s_d = nc.declare_dram_parameter("w1s", [55, 128], bf16, isOutput=False)
    cb2_d = nc.declare_dram_parameter("cb2t", [128, 1], f32, isOutput=False)
    w2p_d = nc.declare_dram_parameter("w2p", [2, 128, 3, 128], bf16, isOutput=False)
    w2s_d = nc.declare_dram_parameter("w2s", [128, 3, 128], bf16, isOutput=False)
    w3t9_d = nc.declare_dram_parameter("w3t9", [9, 128, 256], bf16, isOutput=False)
    w4t9_d = nc.declare_dram_parameter("w4t9", [128, 4, 2, 9, 128], bf16, isOutput=False)
    cb3_d = nc.declare_dram_parameter("cb3t", [128, 2], f32, isOutput=False)
    cb4_d = nc.declare_dram_parameter("cb4t", [128, 4], f32, isOutput=False)
    encw_d = nc.declare_dram_parameter("encwt", [4, 128, HID], f32, isOutput=False)
    encb_d = nc.declare_dram_parameter("encbt", [128, 5], f32, isOutput=False)
    emb_d = nc.declare_dram_parameter("emb", [VOCAB, HID], bf16, isOutput=False)
    attnw_d = nc.declare_dram_parameter("attnwt", [10, 128, HID], bf16, isOutput=False)
    attnb_d = nc.declare_dram_parameter("attnb", [1, HID], bf16, isOutput=False)
    wih_d = nc.declare_dram_parameter("wiht", [20, 128, 10, 128], bf16, isOutput=False)
    whh_d = nc.declare_dram_parameter("whht", [5, 128, 4 * HID], bf16, isOutput=False)
    bgate_d = nc.declare_dram_parameter("bgate", [1, 4 * HID], bf16, isOutput=False)
    fcw_d = nc.declare_dram_parameter("fcwt", [5, 128, VOCAB], bf16, isOutput=False)
    fcb_d = nc.declare_dram_parameter("fcb", [1, VOCAB], bf16, isOutput=False)
    bsel_d = nc.declare_dram_parameter("bsel", [BL, NTOK], f32, isOutput=False)
    # logits stored transposed + group-blocked: [group, vocab_row, chunk, tok]
    NVC = 80          # vocab chunks
    VC = VOCAB // NVC  # 125 vocab rows per chunk
    logits_d = nc.declare_dram_parameter("logits", [10, VC, 8, NTOK], f32,
                                         isOutput=True)

    try:
      with tile.TileContext(nc) as tc:
        # ---------------- persistent constants ----------------
        cpool = tc.alloc_tile_pool(name="const", bufs=1)
        # pool for all DMA-written tiles: never released mid-kernel so that
        # SBUF zone reuse never makes compute ops wait on DMA queue sems
        dmapool = tc.alloc_tile_pool(name="dmat", bufs=1)
        ident = cpool.tile([128, 128], f32)
        make_identity(nc, ident[:, :])
        identb = cpool.tile([128, 128], bf16)
        make_identity(nc, identb[:, :])
        ones64 = cpool.tile([1, 64], bf16)
        nc.gpsimd.memset(ones64[:, :], 1.0)
        bsel_sb = dmapool.tile([BL, NTOK], f32)
        nc.sync.dma_start(out=bsel_sb[:, :], in_=bsel_d[:, :])
        feat_sb = cpool.tile([128, 4, BL], f32)   # feat.T, K-chunked [128,4] per img

        w1s_sb = dmapool.tile([55, 128], bf16)
        nc.sync.dma_start(out=w1s_sb[:, :], in_=w1s_d[:, :])
        cb2_sb = dmapool.tile([128, 1], f32)
        nc.sync.dma_start(out=cb2_sb[:, :], in_=cb2_d[:, :])
        w2p_sb = dmapool.tile([128, 2, 3, 128], bf16)
        nc.sync.dma_start(out=w2p_sb[:, :, :, :],
                          in_=w2p_d[:, :, :, :].rearrange("i p t o -> p i t o"))
        w2s_sb = dmapool.tile([128, 3, 128], bf16)
        nc.sync.dma_start(out=w2s_sb[:, :, :], in_=w2s_d[:, :, :])
        w3_sb = dmapool.tile([128, 9, 256], bf16)
        nc.sync.dma_start(out=w3_sb[:, :, :], in_=w3t9_d[:, :, :].rearrange("t p o -> p t o"))
        cb3_sb = dmapool.tile([128, 2], f32)
        nc.sync.dma_start(out=cb3_sb[:, :], in_=cb3_d[:, :])
        cb4_sb = dmapool.tile([128, 4], f32)
        nc.sync.dma_start(out=cb4_sb[:, :], in_=cb4_d[:, :])

        # ---------------- conv tower ----------------
        # per-image padded intermediates; only borders need zeroing (interiors
        # are fully rewritten). x2 tiles live in their own pool so their SBUF
        # frees right after conv2(im1), making room to prefetch w_ih early.
        spool = tc.alloc_tile_pool(name="seq", bufs=1)
        padpool = tc.alloc_tile_pool(name="pads", bufs=1)
        x2pool = tc.alloc_tile_pool(name="x2p", bufs=1)
        # staging: [im0ch | im1ch] padded pool1 output + an extra always-zero
        # row 114 so the row+1-shifted dup copies need no edge memset
        x2s = x2pool.tile([128, 115, 114], bf16)
        nc.vector.memset(x2s[:, 0, :], 0.0)
        nc.vector.memset(x2s[:, 113:115, :], 0.0)
        nc.vector.memset(x2s[:, :, 0], 0.0)
        nc.vector.memset(x2s[:, :, 113], 0.0)
        x2im = []
        for im in range(BL):
            x2t = x2pool.tile([128, 114, 114], bf16)
            x2im.append(x2t)
        x3im = [None, None]
        x4im = [None, None]

        # ---- conv1 (3->64), both images stacked on partitions (M=64ch x 2im),
        # im2col K=54 + ones-row bias; pool chain: DVE xmax (psum evict),
        # Pool rowmax + relu ----
        _sc1 = nc.enter_named_scope("conv1", False)[0]
        c1pool = tc.alloc_tile_pool(name="c1", bufs=4)
        c1psum = tc.alloc_tile_pool(name="c1p", bufs=4, space="PSUM")
        R = 16
        for ch in range(224 // R):
            Y = R * ch
            rh = dmapool.tile([55, R * 224], bf16, tag="rh", bufs=2)
            nc.sync.dma_start(out=rh[:, :],
                              in_=img_d[:, Y * 224:(Y + R) * 224])
            rhv = rh.rearrange("p (r x) -> p r x", x=224)
            for q in range(4):
                ps = c1psum.tile([128, 2, 448], f32, padded_shape=[128, 2, 512],
                                 tag="ps")
                for s in range(2):
                    r0 = q * 4 + s * 2
                    mm(out=ps[:, s, :], lhsT=w1s_sb[:, :],
                       rhs=rhv[:, r0:r0 + 2, :], start=True, stop=True)
                # relu(maxpool) == max(even, relu(odd), rows): ACT relu-evicts
                # only the odd columns; DVE maxes them against the raw PSUM
                # evens (xm >= 0 always), then row-pools into x2s.
                a1 = c1pool.tile([128, 2, 224], bf16, tag="a1")
                nc.scalar.activation(a1[:, :, :], ps[:, :, 1:448:2], AF.Relu)
                xm = c1pool.tile([128, 2, 224], bf16, tag="xm")
                nc.vector.tensor_tensor(out=xm[:, :, :],
                                        in0=ps[:, :, 0:448:2],
                                        in1=a1[:, :, :], op=ALU.max)
                oy = (R * ch + 4 * q) // 2
                nc.vector.tensor_tensor(out=x2s[:, oy + 1:oy + 3, 1:113],
                                        in0=xm[:, :, 0:112],
                                        in1=xm[:, :, 112:224], op=ALU.max)
        w4_sb = dmapool.tile([128, 4, 2, 9, 128], bf16)
        nc.sync.dma_start(out=w4_sb[:, :, :, :, :], in_=w4t9_d[:, :, :, :, :])
        encw_sb = dmapool.tile([128, 4, HID], f32)
        nc.sync.dma_start(out=encw_sb[:, :, :], in_=encw_d[:, :, :].rearrange("k p o -> p k o"))
        encb_sb = dmapool.tile([128, 5], f32)
        nc.sync.dma_start(out=encb_sb[:, :], in_=encb_d[:, :])
        attnw_sb = dmapool.tile([128, 10, HID], bf16)
        nc.sync.dma_start(out=attnw_sb[:, :, :],
                          in_=attnw_d[:, :, :].rearrange("k p o -> p k o"))
        attnb_sb = dmapool.tile([1, HID], bf16)
        nc.sync.dma_start(out=attnb_sb[:, :], in_=attnb_d[:, :])
        whh_sb = dmapool.tile([128, 5, 4 * HID], bf16)
        nc.sync.dma_start(out=whh_sb[:, :, :],
                          in_=whh_d[:, :, :].rearrange("k p o -> p k o"))
        bgate_sb = dmapool.tile([1, 4 * HID], bf16)
        nc.sync.dma_start(out=bgate_sb[:, :], in_=bgate_d[:, :])
        idx_sb = dmapool.tile([NTOK, 1], i32)
        nc.sync.dma_start(out=idx_sb[:, :], in_=caps_d[:, :])
        e_sb = dmapool.tile([NTOK, HID], bf16)
        nc.gpsimd.indirect_dma_start(
            out=e_sb[:, :], out_offset=None,
            in_=emb_d[:, :],
            in_offset=bass.IndirectOffsetOnAxis(ap=idx_sb[:, :1], axis=0),
        )
        c1psum.release()
        c1pool.release()
        # assemble per-image tap-pair layouts from the staging tile:
        # x2im0 = [im0 | im0 shifted+1], x2im1 = [im1 shifted+1 | im1]
        for r0, r1 in ((0, 57), (57, 114)):
            nc.sync.dma_start(out=x2im[0][0:64, r0:r1, :],
                              in_=x2s[0:64, r0:r1, :])
            nc.sync.dma_start(out=x2im[0][64:128, r0:r1, :],
                              in_=x2s[0:64, r0 + 1:r1 + 1, :])
            nc.sync.dma_start(out=x2im[1][64:128, r0:r1, :],
                              in_=x2s[64:128, r0:r1, :])
            nc.sync.dma_start(out=x2im[1][0:64, r0:r1, :],
                              in_=x2s[64:128, r0 + 1:r1 + 1, :])
        nc.leave_named_scope("conv1", _sc1, False)
        mark("conv1")
        NG = 4 * HID // 128   # 20 gate chunks: i=0-4, f=5-9, o=10-14, g=15-19
        wih_sb = None
        for im in range(BL):
            _sc = nc.enter_named_scope(f"conv_im{im}", False)[0]

            # ---- conv2 (64->128): 3 K=128 tap-pairs + 3 K=64 singles ----
            x3t = padpool.tile([128, 58, 58], bf16)
            nc.vector.memset(x3t[:, :, :], 0.0)
            x3im[im] = x3t
            x4t = padpool.tile([128, 2, 30, 30], bf16)
            nc.vector.memset(x4t[:, :, :, :], 0.0)
            x4im[im] = x4t
            c2psum = tc.alloc_tile_pool(name=f"c2p_{im}", bufs=3, space="PSUM")
            c2pool = tc.alloc_tile_pool(name=f"c2_{im}", bufs=2)
            sb = 0 if im == 0 else 64   # singles partition base for this image
            for tl in range(14):  # 8 output rows per tile
                ps = c2psum.tile([128, 2, 448], f32, padded_shape=[128, 2, 512], tag="ps")
                for s in range(2):
                    y0 = tl * 8 + s * 4
                    for kx in range(3):
                        mm(
                            out=ps[:, s, :], lhsT=w2p_sb[:, im, kx, :],
                            rhs=x2im[im][:, y0:y0 + 4, kx:kx + 112],
                            start=(kx == 0), stop=False,
                        )
                    for kx in range(3):
                        mm(
                            out=ps[:, s, :], lhsT=w2s_sb[sb:sb + 64, kx, :],
                            rhs=x2im[im][sb:sb + 64, y0 + 2:y0 + 6, kx:kx + 112],
                            start=False, stop=(kx == 2),
                        )
                a2 = c2pool.tile([128, 8, 112], bf16, tag="a2")
                nc.scalar.activation(
                    a2.rearrange("p (a y) x -> p a y x", a=2),
                    ps.rearrange("p a (y x) -> p a y x", x=112),
                    AF.Relu, bias=cb2_sb[:, 0:1])
                t2 = c2pool.tile([128, 8, 56], bf16, tag="t2")
                nc.vector.tensor_tensor(
                    out=t2[:, :, :], in0=a2[:, :, 0:112:2], in1=a2[:, :, 1:112:2],
                    op=ALU.max,
                )
                nc.vector.tensor_tensor(
                    out=x3im[im][:, tl * 4 + 1:tl * 4 + 5, 1:57],
                    in0=t2[:, 0:8:2, :], in1=t2[:, 1:8:2, :],
                    op=ALU.max,
                )
            c2psum.release()
            c2pool.release()
            if im == BL - 1:
                # x2 SBUF is dead: free it and prefetch the full w_ih under
                # the remaining conv3/conv4 compute
                x2pool.release()
                wihpool = tc.alloc_tile_pool(name="wihp", bufs=1)
                wih_sb = wihpool.tile([128, NG, 10, 128], bf16)
                for g in range(4):
                    nc.sync.dma_start(
                        out=wih_sb[:, 5 * g:5 * (g + 1), :, :],
                        in_=wih_d[5 * g:5 * (g + 1), :, :, :].rearrange(
                            "m p k o -> p m k o"))

            # ---- conv3 (128->256) K=128, bias via ACT evict, pool -> x4_pad ----
            c3psum = tc.alloc_tile_pool(name=f"c3p_{im}", bufs=3, space="PSUM")
            c3pool = tc.alloc_tile_pool(name=f"c3_{im}", bufs=2)
            for m in range(2):
                for tl in range(7):  # 8 output rows per tile
                    ps = c3psum.tile([128, 448], f32, padded_shape=[128, 512], tag="ps")
                    y0 = tl * 8
                    for ky in range(3):
                        for kx in range(3):
                            tap = ky * 3 + kx
                            rhs = x3im[im][:, y0 + ky:y0 + ky + 8, kx:kx + 56]
                            mm(
                                out=ps[:, :],
                                lhsT=w3_sb[:, tap, 128 * m:128 * (m + 1)],
                                rhs=rhs,
                                start=(tap == 0), stop=(tap == 8),
                            )
                    a3 = c3pool.tile([128, 8, 56], bf16, tag="a3")
                    nc.scalar.activation(
                        a3[:, :, :],
                        ps.rearrange("p (y x) -> p y x", x=56),
                        AF.Relu, bias=cb3_sb[:, m:m + 1])
                    t3 = c3pool.tile([128, 8, 28], bf16, tag="t3")
                    nc.vector.tensor_tensor(
                        out=t3[:, :, :], in0=a3[:, :, 0:56:2], in1=a3[:, :, 1:56:2],
                        op=ALU.max,
                    )
                    nc.vector.tensor_tensor(
                        out=x4im[im][:, m, tl * 4 + 1:tl * 4 + 5, 1:29],
                        in0=t3[:, 0:8:2, :], in1=t3[:, 1:8:2, :],
                        op=ALU.max,
                    )
            c3psum.release()
            c3pool.release()

            # ---- conv4 (256->512) K=256 (2 chunks), no pool; mean via accum_out ----
            ipool = tc.alloc_tile_pool(name=f"img{im}", bufs=1)
            c4psum = tc.alloc_tile_pool(name=f"c4p_{im}", bufs=3, space="PSUM")
            c4pool = tc.alloc_tile_pool(name=f"c4_{im}", bufs=2)
            msum = ipool.tile([128, 4, 2], f32)
            for m in range(4):
                w4m = w4_sb[:, m]
                ps = c4psum.tile([128, 2, 392], f32, padded_shape=[128, 2, 512], tag="ps")
                for s in range(2):
                    y0 = s * 14
                    first = True
                    for ky in range(3):
                        for kx in range(3):
                            tap = ky * 3 + kx
                            for k2 in range(2):
                                rhs = x4im[im][:, k2, y0 + ky:y0 + ky + 14, kx:kx + 28]
                                mm(
                                    out=ps[:, s, :],
                                    lhsT=w4m[:, k2, tap, :],
                                    rhs=rhs,
                                    start=first, stop=(tap == 8 and k2 == 1),
                                )
                                first = False
                a4 = c4pool.tile([128, 2, 392], bf16, tag="a4")
                for s in range(2):
                    nc.scalar.activation(a4[:, s, :], ps[:, s, :], AF.Relu,
                                         bias=cb4_sb[:, m:m + 1],
                                         accum_out=msum[:, m, s:s + 1])
            c4psum.release()
            c4pool.release()
            # feat.T[:, m] = (msum[:,m,0] + msum[:,m,1]) / 784
            tmpf = ipool.tile([128, 4], f32)
            nc.vector.tensor_tensor(out=tmpf[:, :], in0=msum[:, :, 0], in1=msum[:, :, 1],
                                    op=ALU.add)
            nc.vector.tensor_scalar_mul(feat_sb[:, :, im], tmpf[:, :], 1.0 / 784.0)
            ipool.release()
            nc.leave_named_scope(f"conv_im{im}", _sc, False)
            mark(f"conv_im{im}")

        if upto == "conv":
            raise _PhaseExit(tc)

        # ---------------- encoder linear: memory.T = enc_w @ feat.T + enc_b ----------------
        _sc_ea = nc.enter_named_scope("enc_attn", False)[0]
        scpool = tc.alloc_tile_pool(name="scratch", bufs=1)
        p1psum = tc.alloc_tile_pool(name="p1ps", bufs=1, space="PSUM")
        memT_ps = p1psum.tile([128, 5, BL], f32)
        for m in range(5):
            for k in range(4):
                nc.tensor.matmul(
                    out=memT_ps[:, m, :],
                    lhsT=encw_sb[:, k, 128 * m:128 * (m + 1)],
                    rhs=feat_sb[:, k, :],
                    start=(k == 0), stop=(k == 3),
                )
        memT_sb = spool.tile([128, 5, BL], f32)
        for m in range(5):
            nc.vector.tensor_scalar_add(memT_sb[:, m, :], memT_ps[:, m, :],
                                        encb_sb[:, m:m + 1])
        # memory non-transposed [2, 640]
        mem_ps = p1psum.tile([BL, HID], f32)
        for m in range(5):
            nc.tensor.transpose(out=mem_ps[:, 128 * m:128 * (m + 1)],
                                in_=memT_sb[:, m, :], identity=ident[:, :])
        mem_sb = scpool.tile([BL, HID], f32)
        nc.scalar.copy(mem_sb[:, :], mem_ps[:, :])

        # memory broadcast to all tokens [64, 640] via bsel matmul
        mexp_ps = p1psum.tile([NTOK, HID], f32)
        for n in range(2):
            sl = slice(512 * n, min(HID, 512 * (n + 1)))
            nc.tensor.matmul(out=mexp_ps[:, sl], lhsT=bsel_sb[:, :], rhs=mem_sb[:, sl],
                             start=True, stop=True)
        mexp_sb = scpool.tile([NTOK, HID], f32)
        nc.scalar.copy(mexp_sb[:, :], mexp_ps[:, :])
        p1psum.release()
        p1bpsum = tc.alloc_tile_pool(name="p1bps", bufs=1, space="PSUM")

        # fusedT [128, 10, 64]: chunks 0-4 = e.T ; 5-9 = memory.T broadcast
        fusedT_pse = p1bpsum.tile([128, 5, NTOK], bf16)
        for k in range(5):
            nc.tensor.transpose(out=fusedT_pse[:, k, :],
                                in_=e_sb[:, 128 * k:128 * (k + 1)],
                                identity=identb[0:64, 0:64])
        fusedT_psm = p1bpsum.tile([128, 5, NTOK], f32)
        for m in range(5):
            nc.tensor.matmul(out=fusedT_psm[:, m, :],
                             lhsT=mem_sb[:, 128 * m:128 * (m + 1)],
                             rhs=bsel_sb[:, :], start=True, stop=True)
        fusedT_sb = spool.tile([128, 10, NTOK], bf16)
        nc.scalar.copy(fusedT_sb[:, 0:5, :], fusedT_pse[:, :, :])
        nc.scalar.copy(fusedT_sb[:, 5:10, :], fusedT_psm[:, :, :])

        # ---------------- attention (batched over all tokens) ----------------
        attnw_sb = dmapool.tile([128, 10, HID], bf16)
        nc.sync.dma_start(out=attnw_sb[:, :, :],
                          in_=attnw_d[:, :, :].rearrange("k p o -> p k o"))
        attnb_sb = dmapool.tile([1, HID], bf16)
        nc.sync.dma_start(out=attnb_sb[:, :], in_=attnb_d[:, :])

        attn_ps = p1bpsum.tile([NTOK, HID], f32)
        for n in range(2):
            sl = slice(512 * n, min(HID, 512 * (n + 1)))
            for k in range(10):
                mm(out=attn_ps[:, sl], lhsT=fusedT_sb[:, k, :],
                   rhs=attnw_sb[:, k, sl], start=(k == 0), stop=False)
            mm(out=attn_ps[:, sl], lhsT=ones64[:, :],
               rhs=attnb_sb[:, sl], start=False, stop=True)
        # softmax over free dim, then context = softmax * memory
        nmx_sb = scpool.tile([NTOK, 1], f32)
        nc.vector.reduce_max(out=nmx_sb[:, :], in_=attn_ps[:, :], axis=AX.X,
                             negate=True)
        ex_sb = scpool.tile([NTOK, HID], f32)
        ssum_sb = scpool.tile([NTOK, 1], f32)
        nc.scalar.activation(ex_sb[:, :], attn_ps[:, :], AF.Exp,
                             bias=nmx_sb[:, 0:1], accum_out=ssum_sb[:, 0:1])
        rcp_sb = scpool.tile([NTOK, 1], f32)
        nc.vector.reciprocal(rcp_sb[:, :], ssum_sb[:, :])
        ctx_sb = scpool.tile([NTOK, HID], bf16)
        nc.vector.tensor_scalar_mul(ctx_sb[:, :], ex_sb[:, :], rcp_sb[:, 0:1])
        nc.vector.tensor_tensor(out=ctx_sb[:, :], in0=ctx_sb[:, :], in1=mexp_sb[:, :],
                                op=ALU.mult)
        ctxT_ps = p1bpsum.tile([128, 5, NTOK], bf16)
        for k in range(5):
            nc.tensor.transpose(out=ctxT_ps[:, k, :],
                                in_=ctx_sb[:, 128 * k:128 * (k + 1)],
                                identity=identb[0:64, 0:64])
        ctxT_sb = spool.tile([128, 5, NTOK], bf16)
        nc.scalar.copy(ctxT_sb[:, :, :], ctxT_ps[:, :, :])
        p1bpsum.release()
        scpool.release()

        p2psum = tc.alloc_tile_pool(name="p2ps", bufs=1, space="PSUM")
        PT_ps = p2psum.tile([128, NG, NTOK], f32)
        for m in range(NG):
            for k in range(10):
                rhsT = fusedT_sb[:, k, :] if k < 5 else ctxT_sb[:, k - 5, :]
                mm(out=PT_ps[:, m, :], lhsT=wih_sb[:, m, k, :],
                   rhs=rhsT, start=(k == 0), stop=False)
            mm(out=PT_ps[:, m, :], lhsT=bgate_sb[:, 128 * m:128 * (m + 1)],
               rhs=ones64[:, :], start=False, stop=True)
        PT_sb = spool.tile([128, NG, NTOK], bf16)
        nc.scalar.copy(PT_sb[:, :, :], PT_ps[:, :, :])
        p2psum.release()
        wihpool.release()
        padpool.release()
        nc.leave_named_scope("enc_attn", _sc_ea, False)
        mark("enc_attn")

        if upto == "pre":
            raise _PhaseExit(tc)
        _sc_ls = nc.enter_named_scope("lstm", False)[0]
        # ---------------- LSTM recurrence (transposed: gates.T on partitions) ----------------
        outsT_sb = spool.tile([128, 5, NTOK], bf16)   # h.T for every step
        c_sb = spool.tile([128, 5, BL], f32)

        # FC weight stream: allocate + DMA before the LSTM so transfers overlap it
        fcwpool = tc.alloc_tile_pool(name="fcw", bufs=1)
        lpsum = tc.alloc_tile_pool(name="lstm_ps", bufs=2, space="PSUM")
        lsp = tc.alloc_tile_pool(name="lstm_sb", bufs=2)
        CH = 1000
        fws = []
        for j in range(VOCAB // CH):
            fw = fcwpool.tile([128, 5, CH], bf16, tag="fw", bufs=10)
            nc.sync.dma_start(out=fw[:, :, :],
                              in_=fcw_d[:, :, CH * j:CH * (j + 1)].rearrange(
                                  "k p o -> p k o"))
            fcb_sb = fcwpool.tile([1, CH], bf16, tag="fcb", bufs=2)
            nc.sync.dma_start(out=fcb_sb[:, :], in_=fcb_d[:, CH * j:CH * (j + 1)])
            fws.append((fw, fcb_sb))
        for t in range(T):
            tt = slice(BL * t, BL * (t + 1))
            if t == 0:
                gsum = PT_sb
                gt = tt
            else:
                gt = slice(0, BL)
                gatesT_ps = lpsum.tile([128, NG, BL], f32, tag="gates")
                # seed the whole gate tile (one PSUM bank) with P_t in a single
                # identity matmul -- no h dependency, so it overlaps the prior
                # step's elementwise; W_hh @ h then accumulates on top
                mm(out=gatesT_ps[:, :, :], lhsT=identb[:, :],
                   rhs=PT_sb[:, :, tt], start=True, stop=False)
                for m in range(NG):
                    for k in range(5):
                        mm(out=gatesT_ps[:, m, :],
                           lhsT=whh_sb[:, k, 128 * m:128 * (m + 1)],
                           rhs=outsT_sb[:, k, BL * (t - 1):BL * t],
                           start=False, stop=(m == NG - 1 and k == 4))
                gsum = gatesT_ps
            # one sigmoid covers every gate: host doubled the g-gate rows, so
            # tanh(g) = 2*sigmoid(2g) - 1 comes from a cheap DVE affine
            sig = lsp.tile([128, NG, BL], f32, tag="sig")
            nc.scalar.activation(sig[:, :, :], gsum[:, 0:NG, gt], AF.Sigmoid)
            tg = lsp.tile([128, 5, BL], f32, tag="tg")
            nc.vector.tensor_scalar(tg[:, :, :], sig[:, 15:20, :], 2.0, -1.0,
                                    ALU.mult, ALU.add)
            # f*c first: it only needs sig, so DVE overlaps ACT's tanh
            if t > 0:
                nc.vector.tensor_tensor(out=c_sb[:, :, :], in0=sig[:, 5:10, :],
                                        in1=c_sb[:, :, :], op=ALU.mult)
            ig = lsp.tile([128, 5, BL], f32, tag="ig")
            nc.vector.tensor_tensor(out=ig[:, :, :], in0=sig[:, 0:5, :],
                                    in1=tg[:, :, :], op=ALU.mult)
            if t > 0:
                nc.vector.tensor_tensor(out=c_sb[:, :, :], in0=c_sb[:, :, :],
                                        in1=ig[:, :, :], op=ALU.add)
            else:
                nc.vector.tensor_copy(out=c_sb[:, :, :], in_=ig[:, :, :])
            thc = lsp.tile([128, 5, BL], f32, tag="thc")
            nc.scalar.activation(thc[:, :, :], c_sb[:, :, :], AF.Tanh)
            nc.vector.tensor_tensor(out=outsT_sb[:, :, tt], in0=sig[:, 10:15, :],
                                    in1=thc[:, :, :], op=ALU.mult)
        lsp.release()
        lpsum.release()
        nc.leave_named_scope("lstm", _sc_ls, False)
        mark("lstm")

        if upto == "lstm":
            raise _PhaseExit(tc)
        _sc_fc = nc.enter_named_scope("fc", False)[0]
        # -------- FC to vocab (transposed): logits.T = fc_w @ outs.T + fc_b --------
        # vocab rows on partitions (M=125), tokens streaming (N=64); PSUM -> DRAM.
        fpsum = tc.alloc_tile_pool(name="fc_ps", bufs=2, space="PSUM")
        NCHK = CH // VC   # 8 vocab chunks per CH group = one full PSUM bank
        for j in range(VOCAB // CH):
            fw, fcb_sb = fws[j]
            ps = fpsum.tile([VC, NCHK, NTOK], f32, tag="ps")
            for s in range(NCHK):
                for k in range(5):
                    mm(out=ps[:, s, :], lhsT=fw[:, k, VC * s:VC * (s + 1)],
                       rhs=outsT_sb[:, k, :],
                       start=(k == 0), stop=False)
                mm(out=ps[:, s, :], lhsT=fcb_sb[:, VC * s:VC * (s + 1)],
                   rhs=ones64[:, :], start=False, stop=True)
            lo = spool.tile([VC, NCHK, NTOK], f32, tag="lo", bufs=2)
            if j % 2 == 0:
                nc.scalar.copy(lo[:, :, :], ps[:, :, :])
            else:
                nc.vector.tensor_copy(out=lo[:, :, :], in_=ps[:, :, :])
            nc.sync.dma_start(out=logits_d[j, :, :, :], in_=lo[:, :, :])
        fpsum.release()
        fcwpool.release()
        nc.leave_named_scope("fc", _sc_fc, False)
        mark("fc")
        spool.release()
        dmapool.release()
        cpool.release()
    except _PhaseExit:
        pass

    nc.finalize()
    return nc


def _prep_shared(inputs):
    """Host-side weight layout prep (shared across cores)."""
    import ml_dtypes
    bf = ml_dtypes.bfloat16
    f = np.float32
    perm = _gate_perm()
    w1 = inputs["cw1"].astype(f)
    w1b = w1.transpose(2, 3, 1, 0).reshape(27, 64)
    # block-diagonal stacked-images conv1 weight + bias row (K=55)
    w1s = np.zeros((55, 128), f)
    w1s[0:27, 0:64] = w1b
    w1s[27:54, 64:128] = w1b
    w1s[54, 0:64] = inputs["cb1"].astype(f)
    w1s[54, 64:128] = inputs["cb1"].astype(f)
    cb2t = inputs["cb2"].astype(f).reshape(128, 1).copy()
    w2t9 = inputs["cw2"].astype(f).transpose(2, 3, 1, 0).reshape(9, 64, 128)
    # tap pairs per image: im0 = [ky0; ky1] (dup half holds row+1), im1 = [ky1; ky0]
    w2p0 = np.concatenate([w2t9[0:3], w2t9[3:6]], axis=1)   # [3, 128, 128]
    w2p1 = np.concatenate([w2t9[3:6], w2t9[0:3]], axis=1)
    w2p = np.stack([w2p0, w2p1]).transpose(0, 2, 1, 3).copy()  # [2, 128, 3, 128]
    # singles (ky=2) duplicated in both partition halves
    w2s = np.concatenate([w2t9[6:9], w2t9[6:9]], axis=1).transpose(1, 0, 2).copy()
    w3t9 = inputs["cw3"].astype(f).transpose(2, 3, 1, 0).reshape(9, 128, 256)
    # [ky*kx=9, k2=2, 128, 512] -> [p=128, m=4, k2=2, tap=9, o=128]
    w4t9 = (inputs["cw4"].astype(f).transpose(2, 3, 1, 0).reshape(9, 2, 128, 4, 128)
            .transpose(2, 3, 1, 0, 4).copy())
    cb3t = inputs["cb3"].astype(f).reshape(2, 128).T.copy()
    cb4t = inputs["cb4"].astype(f).reshape(4, 128).T.copy()
    encwt = inputs["enc_w"].astype(f).T.reshape(4, 128, HID).copy()
    encbt = inputs["enc_b"].astype(f).reshape(5, 128).T.copy()
    attnwt = inputs["attn_w"].astype(f).T.reshape(10, 128, HID).copy()
    attnb = inputs["attn_b"].astype(f)[None, :]
    wih = inputs["w_ih"].astype(f)[perm]
    whh = inputs["w_hh"].astype(f)[perm]
    # g-gate rows x2 so tanh(g) = 2*sigmoid(2g) - 1 (exact bf16 scale)
    wih[1920:2560] *= 2.0
    whh[1920:2560] *= 2.0
    # wih.T [2H, 4H] -> [m=20, p=128(k), kk=10, g=128]
    wiht = wih.T.reshape(10, 128, 20, 128).transpose(2, 1, 0, 3).copy()
    whht = whh.T.reshape(5, 128, 4 * HID).copy()
    bgate = (inputs["b_ih"].astype(f) + inputs["b_hh"].astype(f))[perm][None, :].copy()
    bgate[:, 1920:2560] *= 2.0
    fcwt = inputs["fc_w"].astype(f).T.reshape(5, 128, VOCAB).copy()
    fcb = inputs["fc_b"].astype(f)[None, :]
    bsel = np.zeros((BL, NTOK), f)
    for p in range(NTOK):
        bsel[p % BL, p] = 1.0
    return dict(w1s=w1s.astype(bf), cb2t=cb2t,
                w2p=w2p.astype(bf), w2s=w2s.astype(bf),
                w3t9=w3t9.astype(bf), w4t9=w4t9.astype(bf),
                cb3t=cb3t, cb4t=cb4t, encwt=encwt, encbt=encbt,
                attnwt=attnwt.astype(bf), attnb=attnb.astype(bf),
                wiht=wiht.astype(bf), whht=whht.astype(bf), bgate=bgate.astype(bf),
                fcwt=fcwt.astype(bf), fcb=fcb.astype(bf), bsel=bsel,
                emb=inputs["emb"].astype(f).astype(bf))


def _make_in_maps(inputs):
    shared = _prep_shared(inputs)
    images = np.asarray(inputs["images"], np.float32)
    captions = np.asarray(inputs["captions"])

    import ml_dtypes
    imgp = np.zeros((16, 3, 226, 226), np.float32)
    imgp[:, :, 1:225, 1:225] = images
    s = imgp.strides
    win = np.lib.stride_tricks.as_strided(
        imgp, shape=(16, 3, 3, 3, 224, 224),
        strides=(s[0], s[1], s[2], s[3], s[2], s[3]))
    # rows (ky, kx, c) to match w1 layout
    imcol = win.transpose(0, 2, 3, 1, 4, 5).reshape(16, 27, 224 * 224)
    imgp = imcol.astype(ml_dtypes.bfloat16)
    ones_row = np.ones((1, 224 * 224), ml_dtypes.bfloat16)
    in_maps = []
    for c in range(NCORES):
        caps = captions[BL * c:BL * (c + 1)].astype(np.int64).T.reshape(NTOK, 1)
        m = dict(shared)
        m["img"] = np.concatenate(
            [imgp[BL * c], imgp[BL * c + 1], ones_row], axis=0)
        m["caps"] = caps.astype(np.int32)
        in_maps.append(m)
    return in_maps


def kernel(**inputs):
    from concourse.bass_utils import run_bass_kernel_spmd

    if "nc" not in _NC_CACHE:
        _NC_CACHE["nc"] = build_bass()
    nc = _NC_CACHE["nc"]

    in_maps = _make_in_maps(inputs)
    res = run_bass_kernel_spmd(nc, in_maps, list(range(NCORES)))
    # logits come back as [j=10, v=125, s=8, tok]: vocab index = j*1000+s*125+v
    out = np.concatenate(
        [res.results[c]["logits"].transpose(0, 2, 1, 3)
             .reshape(VOCAB, T, BL).transpose(2, 1, 0)
         for c in range(NCORES)], axis=0)
    return out

